# revision 54
# baseline (speedup 1.0000x reference)
"""Trainium2 Bass kernel for a Bayesian MLP (local reparameterization trick).

Reference computation (per sample s of S=10):
    h1 = leaky_relu(x @ W1m + sqrt(x^2 @ W1v + 1e-12) * eps1_s)         [B, 512]
    h2 = leaky_relu(h1a @ W2m + sqrt(h1a^2 @ W2v + 1e-12) * eps2_s)     (h1a = [h1, 1])
    h3 = leaky_relu(h2a @ W3m + sqrt(h2a^2 @ W3v + 1e-12) * eps3_s)
    out = log_softmax(h3a @ W4m + sqrt(h3a^2 @ W4v + 1e-12) * eps4_s)   [B, 10]

Distribution: data-parallel over the batch axis, B=2048 -> 8 cores x 256 rows.
Small variational parameters replicated on every core.

v2 design notes:
  * activations [feature on partitions, (pair, sample, batch) free]; all ten
    samples live in one free axis of 5*512 = 2560 per feature block
  * mean matmuls in bf16 (stationary reused across the 5 sample-pairs)
  * variance matmuls in fp8 e4m3 with DoubleRow (K=256 per pass):
    hq = 8*h^2 (fp8), wv' = 256*v (fp8), descaled inside the ACT sqrt
  * elementwise work spread over ACT/DVE/Pool; big [128, 2560] ops where
    PSUM granularity allows
  * log-softmax phase at the end (single activation-table switch); the
    mean bias of layer 4 rides the Exp bias and a [2,C] stationary trick
"""

import sys
import os

for _p in ("/opt/trn_rl_repo",):
    if _p not in sys.path and os.path.isdir(_p):
        sys.path.insert(0, _p)

import numpy as np
import ml_dtypes

import concourse.bass as bass
import concourse.bacc as bacc
import concourse.mybir as mybir
from concourse import tile
from concourse.bass_utils import run_bass_kernel_spmd

F32 = mybir.dt.float32
F32R = mybir.dt.float32r
BF16 = mybir.dt.bfloat16
FP8 = mybir.dt.float8e4
AF = mybir.ActivationFunctionType
ALU = mybir.AluOpType
DR = mybir.MatmulPerfMode.DoubleRow


def _register_prelu_add():
    """Fused u = in0 + in1 + s0; out = max(0.01*u, u) as ONE DVE op.

    Replaces the separate tensor add and prelu passes of the local
    reparameterization chain (in1 may live in PSUM)."""
    import concourse.dve_ops as D
    from concourse.dve_spec import Spec, Src0, Src1, C0, C2, maxx, lower
    from concourse.dve_uop import DveOpSpec

    name = "PRELU_ADD_ANT"
    if name in D._SUB_OPCODE_FOR_NAME:
        for o in D.OPS:
            if o.name == name:
                return o
    _b = Src0 + Src1 + C0
    spec = Spec(
        body=maxx(_b * C2, _b),
        reference=lambda in0, in1, s0, s1, imm2: np.maximum(
            (in0.astype(np.float32) + in1 + s0) * imm2,
            in0.astype(np.float32) + in1 + s0),
    )
    opcode = D._CUSTOM_DVE_ROW_BASE + len(D.OPS)
    assert opcode < 0x20
    shas = {}
    for ver in ("v3", "v4"):
        uops = lower(spec, ver=ver)
        shas[ver] = DveOpSpec(name=name, opcode=opcode, uops=uops,
                              rd1_en=True).sha(ver)
    op = D.DveOp(name, spec, subdim=False, uops_sha=shas)
    D.OPS.append(op)
    D._SUB_OPCODE_FOR_NAME[name] = opcode
    return op


PRELU_ADD = _register_prelu_add()

B, D_IN, H, C, S = 2048, 784, 512, 10, 10
N_CORES = 8
BL = B // N_CORES            # 256 rows per core
NP = S // 2                  # 5 sample-pairs
FD = 2 * BL                  # 512 free per pair
FDA = NP * FD                # 2560 free, all pairs
K1 = 7                       # 784 -> 7 chunks of 112... no: 896/128
K1P = 8                      # padded to 8 for fp8 DoubleRow pairing
KH = 4                       # 512/128
FO = 4
SQ_SCALE = 2.0               # hq = SQ_SCALE * h^2 (compile-time, ACT Square)
X2_SCALE = 4.0               # x2q = X2_SCALE * x^2
# The fp8 variance-weight scale WV is chosen at RUNTIME per layer (so that
# constant-v layers land exactly on the fp8 grid); the descale is folded into
# the host-side eps tensors and sqrt-bias APs, so the device sqrt has scale=1.

PAIR_GROUPS = ((0, 1, 2), (3, 4))


def build_program(dbg=False):
    nc = bacc.Bacc("TRN2", target_bir_lowering=False, debug=False)

    # ---- DRAM I/O (per core) ----
    xT_d = nc.dram_tensor("xT", [K1, 128, BL], BF16, kind="ExternalInput")
    x2q_d = nc.dram_tensor("x2q", [128, K1P, BL], FP8, kind="ExternalInput")
    w1m_d = nc.dram_tensor("w1m", [K1, 128, H], BF16, kind="ExternalInput")
    w1vq_d = nc.dram_tensor("w1vq", [128, K1P, H], FP8, kind="ExternalInput")
    w2m_d = nc.dram_tensor("w2m", [128, KH, H], BF16, kind="ExternalInput")
    w2vq_d = nc.dram_tensor("w2vq", [128, KH, H], FP8, kind="ExternalInput")
    w3m_d = nc.dram_tensor("w3m", [128, KH, H], BF16, kind="ExternalInput")
    w3vq_d = nc.dram_tensor("w3vq", [128, KH, H], FP8, kind="ExternalInput")
    w4m_d = nc.dram_tensor("w4m", [128, KH, C], BF16, kind="ExternalInput")
    w4vq_d = nc.dram_tensor("w4vq", [128, KH, C], FP8, kind="ExternalInput")
    b2mP_d = nc.dram_tensor("b2mP", [128, FO], F32, kind="ExternalInput")
    b3mP_d = nc.dram_tensor("b3mP", [128, FO], F32, kind="ExternalInput")
    b2v_d = nc.dram_tensor("b2v", [128, FO], F32, kind="ExternalInput")
    b3v_d = nc.dram_tensor("b3v", [128, FO], F32, kind="ExternalInput")
    b4v_d = nc.dram_tensor("b4v", [C, 1], F32, kind="ExternalInput")
    b4m_d = nc.dram_tensor("b4m", [C, 1], F32, kind="ExternalInput")
    bneg_d = nc.dram_tensor("bneg", [2, C], F32R, kind="ExternalInput")
    e1_d = nc.dram_tensor("e1", [FO, 128, FDA], BF16, kind="ExternalInput")
    e2_d = nc.dram_tensor("e2", [FO, 128, FDA], BF16, kind="ExternalInput")
    e3_d = nc.dram_tensor("e3", [FO, 128, FDA], BF16, kind="ExternalInput")
    e4_d = nc.dram_tensor("e4", [C, FDA], BF16, kind="ExternalInput")
    ones_row_d = nc.dram_tensor("ones_row_in", [1, FD], F32R, kind="ExternalInput")
    ones10_d = nc.dram_tensor("ones10_in", [C, 1], F32R, kind="ExternalInput")
    out_d = nc.dram_tensor("out", [C, FDA], F32, kind="ExternalOutput")
    if dbg:
        dbg_sig1 = nc.dram_tensor("dbg_sig1", [128, FO * FD], F32,
                                  kind="ExternalOutput")
        dbg_mu1 = nc.dram_tensor("dbg_mu1", [128, FO * FD], BF16,
                                 kind="ExternalOutput")
        dbg_h1 = nc.dram_tensor("dbg_h1", [FO, 128, FDA], BF16,
                                kind="ExternalOutput")
        dbg_hq1 = nc.dram_tensor("dbg_hq1", [2, 128, 2 * FDA], FP8,
                                 kind="ExternalOutput")
        dbg_h2 = nc.dram_tensor("dbg_h2", [FO, 128, FDA], BF16,
                                kind="ExternalOutput")
        dbg_u4 = nc.dram_tensor("dbg_u4", [C, FDA], F32,
                                kind="ExternalOutput")

    def mm(out_ap, lhsT_ap, rhs_ap, start, stop, perf_mode=None):
        nc.tensor.matmul(out_ap, lhsT_ap, rhs_ap, start=start, stop=stop,
                         perf_mode=perf_mode)

    with tile.TileContext(nc) as tc:
        with (
            tc.tile_pool(name="wp", bufs=1) as wp,
            tc.tile_pool(name="sp", bufs=1) as sp,
            tc.tile_pool(name="hp", bufs=1) as hp,
            tc.tile_pool(name="ep", bufs=1) as ep,
            tc.tile_pool(name="tp", bufs=1) as tp,
        ):
            # ---- persistent weights ----
            w1m_t = [wp.tile([128, H], BF16, tag=f"w1m{k}", name=f"w1m{k}")
                     for k in range(K1)]
            w1vq_t = wp.tile([128, K1P, H], FP8, tag="w1vq", name="w1vq")
            xT_t = [wp.tile([128, BL], BF16, tag=f"xT{k}", name=f"xT{k}")
                    for k in range(K1)]
            x2q_t = wp.tile([128, K1P, BL], FP8, tag="x2q", name="x2q")
            w2m_t = wp.tile([128, KH, H], BF16, tag="w2m", name="w2m")
            w2vq_t = wp.tile([128, KH, H], FP8, tag="w2vq", name="w2vq")
            w3m_t = wp.tile([128, KH, H], BF16, tag="w3m", name="w3m")
            w3vq_t = wp.tile([128, KH, H], FP8, tag="w3vq", name="w3vq")
            w4m_t = wp.tile([128, KH, C], BF16, tag="w4m", name="w4m")
            w4vq_t = wp.tile([128, KH, C], FP8, tag="w4vq", name="w4vq")
            b2mP_t = wp.tile([128, FO], F32, tag="b2mP", name="b2mP")
            b3mP_t = wp.tile([128, FO], F32, tag="b3mP", name="b3mP")
            b2v_t = wp.tile([128, FO], F32, tag="b2v", name="b2v")
            b3v_t = wp.tile([128, FO], F32, tag="b3v", name="b3v")
            b4v_t = wp.tile([C, 1], F32, tag="b4v", name="b4v")
            b4m_t = wp.tile([C, 1], F32, tag="b4m", name="b4m")
            bneg_t = wp.tile([2, C], F32R, tag="bneg", name="bneg")
            ones10 = wp.tile([C, 1], F32R, tag="ones10", name="ones10")
            eps12_t = wp.tile([128, 1], F32, tag="eps12", name="eps12")
            nc.vector.memset(eps12_t[:], 1e-12)

            # ---- persistent activations ----
            # si-duplicated layer-1 stats (so L1 ops need no broadcast reads)
            mu1e_t = sp.tile([128, FO * FD], BF16, tag="mu1e", name="mu1e")
            sig1e_t = sp.tile([128, FO * FD], F32, tag="sig1e", name="sig1e")
            u4_all = sp.tile([C, FDA], F32, tag="u4", name="u4")
            e4_t = sp.tile([C, FDA], BF16, tag="e4", name="e4")
            lsem_t = [sp.tile([2, FD], F32R, tag=f"lsem{i}", name=f"lsem{i}")
                      for i in range(2)]

            # ---- weight DMAs ----
            for k in range(K1):
                nc.sync.dma_start(w1m_t[k][:], w1m_d[k])
                nc.sync.dma_start(xT_t[k][:], xT_d[k])
            nc.sync.dma_start(w1vq_t[:], w1vq_d[:])
            nc.sync.dma_start(x2q_t[:], x2q_d[:])
            nc.sync.dma_start(w2m_t[:], w2m_d[:])
            nc.sync.dma_start(w2vq_t[:], w2vq_d[:])
            nc.sync.dma_start(w3m_t[:], w3m_d[:])
            nc.sync.dma_start(w3vq_t[:], w3vq_d[:])
            nc.sync.dma_start(w4m_t[:], w4m_d[:])
            nc.sync.dma_start(w4vq_t[:], w4vq_d[:])
            nc.sync.dma_start(b2mP_t[:], b2mP_d[:])
            nc.sync.dma_start(b3mP_t[:], b3mP_d[:])
            nc.sync.dma_start(b2v_t[:], b2v_d[:])
            nc.sync.dma_start(b3v_t[:], b3v_d[:])
            nc.sync.dma_start(b4v_t[:], b4v_d[:])
            nc.sync.dma_start(b4m_t[:], b4m_d[:])
            nc.sync.dma_start(bneg_t[:], bneg_d[:])
            nc.sync.dma_start(ones10[:], ones10_d[:])
            nc.sync.dma_start(e4_t[:], e4_d[:])
            for i in range(2):
                nc.sync.dma_start(lsem_t[i][1:2, :], ones_row_d[:])

            # eps tiles: tag per fo, double-buffered across layers
            def load_eps(e_d):
                ts = []
                for fo in range(FO):
                    t = ep.tile([128, FDA], BF16, tag=f"e{fo}", name=f"e{fo}",
                                bufs=2)
                    nc.sync.dma_start(t[:], e_d[fo])
                    ts.append(t)
                return ts

            e1_t = load_eps(e1_d)

            # h/hq tiles: tag per fo / kp, double-buffered across layers
            def h_tiles():
                return [hp.tile([128, FDA], BF16, tag=f"h{fo}", name=f"h{fo}",
                                bufs=2) for fo in range(FO)]

            def hq_tiles():
                return [hp.tile([128, 2, FDA], FP8, tag=f"hq{kp}",
                                name=f"hq{kp}", bufs=2) for kp in range(2)]

            with tc.tile_pool(name="ps", bufs=1, space="PSUM") as ps:
                def mu_ps_tile():
                    return ps.tile([128, FD], F32, tag="mu", name="mu", bufs=4)

                def var_ps_tile():
                    return ps.tile([128, 2 * FD], F32, tag="var2", name="var2",
                                   bufs=2)

                # ---------- Phase A + L1, interleaved per feature block -------
                # L1(fo)'s elementwise rides behind phase A's matmuls for the
                # later feature blocks, so the PE-idle L1 zone shrinks.
                h1_t = h_tiles()
                hq1_t = hq_tiles()
                for fo in range(FO):
                    fs = slice(fo * 128, (fo + 1) * 128)
                    es = slice(fo * FD, (fo + 1) * FD)
                    mu_ps = mu_ps_tile()
                    for k in range(K1):
                        mm(mu_ps[:, 0:BL], w1m_t[k][:, fs], xT_t[k][:],
                           start=(k == 0), stop=(k == K1 - 1))
                    var_ps = var_ps_tile()
                    for kp in range(K1P // 2):
                        mm(var_ps[:, 0:BL], w1vq_t[:, 2 * kp:2 * kp + 2, fs],
                           x2q_t[:, 2 * kp:2 * kp + 2, :],
                           start=(kp == 0), stop=(kp == K1P // 2 - 1),
                           perf_mode=DR)
                    for si in range(2):
                        ss = slice(fo * FD + si * BL, fo * FD + (si + 1) * BL)
                        nc.scalar.activation(sig1e_t[:, ss], var_ps[:, 0:BL],
                                             AF.Sqrt, bias=eps12_t[:])
                        nc.vector.tensor_copy(mu1e_t[:, ss], mu_ps[:, 0:BL])
                    t_l = {}
                    for p in range(NP):
                        sl = slice(p * FD, (p + 1) * FD)
                        t_l[p] = tp.tile([128, FD], BF16, tag="t", name="t",
                                         bufs=3)
                        if p < 3:
                            nc.gpsimd.tensor_tensor(t_l[p][:], sig1e_t[:, es],
                                                    e1_t[fo][:, sl], ALU.mult)
                        else:
                            nc.vector.tensor_tensor(t_l[p][:], sig1e_t[:, es],
                                                    e1_t[fo][:, sl], ALU.mult)
                    for p in range(NP):
                        sl = slice(p * FD, (p + 1) * FD)
                        nc.vector._custom_dve(
                            PRELU_ADD, out=h1_t[fo][:, sl], in0=t_l[p][:],
                            in1=mu1e_t[:, es], s0=0.0, imm2=0.01)
                    nc.scalar.activation(hq1_t[fo // 2][:, fo % 2, :],
                                         h1_t[fo][:], AF.Square, bias=0.0,
                                         scale=float(SQ_SCALE ** 0.5))

                e2_t = load_eps(e2_d)

                # ---------- hidden layers ----------
                VAR_PAIRS = ((0, 1), (2, 3), (4,))

                def hidden_layer(h_in, hq_in, e_t, wm_t, wvq_t, bmP_t, bv_t):
                    h_o = h_tiles()
                    hq_o = hq_tiles()
                    for fo in range(FO):
                        fs = slice(fo * 128, (fo + 1) * 128)
                        sig_t = tp.tile([128, FDA], F32, tag="sigf",
                                        name="sig", bufs=2)
                        # var matmuls first: two pairs share one 2-bank tile
                        for vg in VAR_PAIRS:
                            vt = ps.tile([128, 2 * FD], F32, tag="var2",
                                         name="var2", bufs=2)
                            for kp in range(2):
                                for j, p in enumerate(vg):
                                    mm(vt[:, j * FD:(j + 1) * FD],
                                       wvq_t[:, 2 * kp:2 * kp + 2, fs],
                                       hq_in[kp][:, :, p * FD:(p + 1) * FD],
                                       start=(kp == 0), stop=(kp == 1),
                                       perf_mode=DR)
                            w = len(vg) * FD
                            nc.scalar.activation(
                                sig_t[:, vg[0] * FD:vg[0] * FD + w],
                                vt[:, 0:w], AF.Sqrt, bias=bv_t[:, fo:fo + 1])
                        # mean matmuls p-outer (ldw-opt is off anyway), then
                        # the fused mult / prelu-add chain per pair
                        for p in range(NP):
                            sl = slice(p * FD, (p + 1) * FD)
                            mu_p = mu_ps_tile()
                            for k in range(KH):
                                mm(mu_p[:], wm_t[:, k, fs],
                                   h_in[k][:, sl],
                                   start=(k == 0), stop=(k == KH - 1))
                            t_p = tp.tile([128, FD], BF16, tag="t", name="t",
                                          bufs=3)
                            nc.gpsimd.tensor_tensor(
                                t_p[:], sig_t[:, sl], e_t[fo][:, sl], ALU.mult)
                            nc.vector._custom_dve(
                                PRELU_ADD, out=h_o[fo][:, sl], in0=t_p[:],
                                in1=mu_p[:], s0=bmP_t[:, fo:fo + 1],
                                imm2=0.01)
                        nc.scalar.activation(
                            hq_o[fo // 2][:, fo % 2, :], h_o[fo][:],
                            AF.Square, bias=0.0,
                            scale=float(SQ_SCALE ** 0.5))
                    return h_o, hq_o

                if dbg:
                    nc.sync.dma_start(dbg_sig1[:], sig1e_t[:])
                    nc.sync.dma_start(dbg_mu1[:], mu1e_t[:])
                    for fo in range(FO):
                        nc.sync.dma_start(dbg_h1[fo], h1_t[fo][:])
                    for kp in range(2):
                        nc.sync.dma_start(
                            dbg_hq1[kp],
                            hq1_t[kp][:].rearrange("p a b -> p (a b)"))

                h2_t, hq2_t = hidden_layer(h1_t, hq1_t, e2_t, w2m_t, w2vq_t,
                                           b2mP_t, b2v_t)
                if dbg:
                    for fo in range(FO):
                        nc.sync.dma_start(dbg_h2[fo], h2_t[fo][:])
                e3_t = load_eps(e3_d)
                h3_t, hq3_t = hidden_layer(h2_t, hq2_t, e3_t, w3m_t, w3vq_t,
                                           b3mP_t, b3v_t)

                # ---------- L4 ----------
                for p in range(NP):
                    sl = slice(p * FD, (p + 1) * FD)
                    mu_ps = mu_ps_tile()
                    for k in range(KH):
                        mm(mu_ps[0:C, :], w4m_t[:, k, :], h3_t[k][:, sl],
                           start=(k == 0), stop=(k == KH - 1))
                    var_ps = var_ps_tile()
                    for k in range(KH):
                        mm(var_ps[0:C, 0:FD], w4vq_t[:, k, :],
                           hq3_t[k // 2][:, k % 2, sl],
                           start=(k == 0), stop=(k == KH - 1))
                    sig4_t = tp.tile([C, FD], BF16, tag="sig4", name="sig4",
                                     bufs=2)
                    nc.scalar.activation(sig4_t[:], var_ps[0:C, 0:FD], AF.Sqrt,
                                         bias=b4v_t[:])
                    t4_t = tp.tile([C, FD], BF16, tag="t4", name="t4", bufs=2)
                    nc.vector.tensor_tensor(t4_t[:], sig4_t[:], e4_t[:, sl],
                                            ALU.mult)
                    nc.vector.tensor_tensor(u4_all[:, sl], t4_t[:],
                                            mu_ps[0:C, :], ALU.add)

            if dbg:
                nc.sync.dma_start(dbg_u4[:], u4_all[:])

            # ---------- Phase C: log-softmax (exp/ln table) ----------
            # all Exps first, then all Lns -> exactly two ACT table loads
            with tc.tile_pool(name="psC", bufs=1, space="PSUM") as psC:
                ets, sps = [], []
                for p in range(NP):
                    sl = slice(p * FD, (p + 1) * FD)
                    et = tp.tile([C, FD], F32R, tag="et", name="et", bufs=3)
                    nc.scalar.activation(et[:], u4_all[:, sl], AF.Exp,
                                         bias=b4m_t[:])
                    ets.append(et)
                for p in range(NP):
                    s_ps = psC.tile([1, FD], F32, tag="s", name="s", bufs=5)
                    mm(s_ps[:], ones10[:], ets[p][:], start=True, stop=True)
                    sps.append(s_ps)
                for p in range(NP):
                    sl = slice(p * FD, (p + 1) * FD)
                    lsem = lsem_t[p % 2]
                    nc.scalar.activation(lsem[0:1, :], sps[p][:], AF.Ln,
                                         bias=0.0)
                    lseb_ps = psC.tile([C, FD], F32, tag="lseb", name="lseb",
                                       bufs=2)
                    mm(lseb_ps[:], bneg_t[:], lsem[:], start=True, stop=True)
                    o_t = tp.tile([C, FD], F32, tag="o", name="o", bufs=2)
                    nc.vector.tensor_tensor(o_t[:], u4_all[:, sl], lseb_ps[:],
                                            ALU.subtract)
                    nc.sync.dma_start(out_d[:, sl], o_t[:])

    nc.compile()
    return nc


def _pick_wv(v, base):
    """Scale c ~ base such that typical v*c lands exactly on the fp8 grid.
    For constant-v layers this zeroes the systematic quantization bias."""
    f = np.float32
    pos = v[v > 0]
    if pos.size == 0:
        return f(base)
    vm = f(np.median(pos))
    t = vm * f(base)
    q = f(np.asarray(t, ml_dtypes.float8_e4m3fn).astype(np.float32))
    if q <= 0:
        return f(base)
    return f(q / vm)


def prepare_core_inputs(inputs):
    f = np.float32
    bf = ml_dtypes.bfloat16
    f8 = ml_dtypes.float8_e4m3fn
    x = np.asarray(inputs["inputs"], dtype=f)

    w1m_full = np.asarray(inputs["a1_mean"], f)
    w1m = np.zeros((K1 * 128, H), f)
    w1m[:D_IN] = w1m_full
    w1m = w1m.reshape(K1, 128, H).astype(bf)
    s1 = np.asarray(inputs["a1_dropout"], f) * np.asarray(inputs["a1_scale"], f)
    v1f = (s1 * s1).astype(f)
    wv1 = _pick_wv(v1f, 64.0)
    c1 = X2_SCALE * wv1                      # var_ps = c1 * var
    v1 = np.zeros((K1P * 128, H), f)
    v1[:D_IN] = v1f * wv1
    w1vq = np.ascontiguousarray(
        v1.reshape(K1P, 128, H).transpose(1, 0, 2)).astype(f8)

    def hidden_w(mean, scale, dropout):
        m = np.asarray(mean, f)
        sc = (np.asarray(dropout, f) * np.asarray(scale, f)).astype(f)
        v = sc * sc
        wv = _pick_wv(v[:H], 128.0)
        c = np.float32(SQ_SCALE) * wv        # var_ps = c * var
        wm = np.ascontiguousarray(
            m[:H].reshape(KH, 128, H).transpose(1, 0, 2)).astype(bf)
        wvq = np.ascontiguousarray(
            (v[:H] * wv).reshape(KH, 128, H).transpose(1, 0, 2)).astype(f8)
        bmP = np.ascontiguousarray(m[H].reshape(FO, 128).T).astype(f)
        bv = np.ascontiguousarray(
            ((v[H] + np.float32(1e-12)) * c).reshape(FO, 128).T).astype(f)
        return wm, wvq, bmP, bv, c

    w2m, w2vq, b2mP, b2v, c2 = hidden_w(inputs["a2_mean"], inputs["a2_scale"],
                                        inputs["a2_dropout"])
    w3m, w3vq, b3mP, b3v, c3 = hidden_w(inputs["a3_mean"], inputs["a3_scale"],
                                        inputs["a3_dropout"])

    m4 = np.asarray(inputs["a4_mean"], f)
    s4 = np.asarray(inputs["a4_scale"], f)
    v4 = (s4 * s4).astype(f)
    wv4 = _pick_wv(v4[:H], 128.0)
    c4 = np.float32(SQ_SCALE) * wv4
    w4m = np.ascontiguousarray(
        m4[:H].reshape(KH, 128, C).transpose(1, 0, 2)).astype(bf)
    w4vq = np.ascontiguousarray(
        (v4[:H] * wv4).reshape(KH, 128, C).transpose(1, 0, 2)).astype(f8)
    b4m = np.ascontiguousarray(m4[H].reshape(C, 1)).astype(f)
    b4v = np.ascontiguousarray(((v4[H] + np.float32(1e-12)) * c4)
                               .reshape(C, 1)).astype(f)
    bneg = np.empty((2, C), f)
    bneg[0] = 1.0
    bneg[1] = -m4[H]

    shared = dict(w1m=w1m, w1vq=w1vq, w2m=w2m, w2vq=w2vq, w3m=w3m, w3vq=w3vq,
                  w4m=w4m, w4vq=w4vq, b2mP=b2mP, b3mP=b3mP, b2v=b2v, b3v=b3v,
                  b4v=b4v, b4m=b4m, bneg=bneg,
                  ones_row_in=np.ones((1, FD), dtype=f),
                  ones10_in=np.ones((C, 1), dtype=f))

    eps = [np.asarray(inputs[f"eps{i}"], f) for i in (1, 2, 3, 4)]

    es1 = f(1.0 / np.sqrt(c1))
    es2 = f(1.0 / np.sqrt(c2))
    es3 = f(1.0 / np.sqrt(c3))
    es4 = f(1.0 / np.sqrt(c4))

    def eT(e, b0, es):
        # [S, B, H] -> [FO, 128, (p, si, b)], pre-divided by sqrt(var scale)
        ec = e[:, b0:b0 + BL, :] * es                  # [10, BL, 512]
        return np.ascontiguousarray(
            ec.reshape(NP, 2, BL, FO, 128).transpose(3, 4, 0, 1, 2)
            .reshape(FO, 128, FDA)).astype(bf)

    def e4T(e, b0, es):
        ec = e[:, b0:b0 + BL, :] * es                  # [10, BL, C]
        return np.ascontiguousarray(
            ec.reshape(NP, 2, BL, C).transpose(3, 0, 1, 2)
            .reshape(C, FDA)).astype(bf)

    in_maps = []
    for i in range(N_CORES):
        b0 = i * BL
        xT = np.zeros((K1 * 128, BL), dtype=f)
        xT[:D_IN] = x[b0:b0 + BL].T
        x2 = np.zeros((K1P * 128, BL), dtype=f)
        x2[:D_IN] = (x[b0:b0 + BL].T ** 2) * X2_SCALE
        m = dict(shared)
        m["xT"] = np.ascontiguousarray(xT.reshape(K1, 128, BL)).astype(bf)
        m["x2q"] = np.ascontiguousarray(
            x2.reshape(K1P, 128, BL).transpose(1, 0, 2)).astype(f8)
        m["e1"] = eT(eps[0], b0, es1)
        m["e2"] = eT(eps[1], b0, es2)
        m["e3"] = eT(eps[2], b0, es3)
        m["e4"] = e4T(eps[3], b0, es4)
        in_maps.append(m)
    return in_maps


def gather_output(results):
    out = np.empty((S, B, C), dtype=np.float32)
    for i, r in enumerate(results):
        oc = np.asarray(r["out"])  # [C, (p, si, b)]
        oc = oc.reshape(C, NP, 2, BL).transpose(1, 2, 3, 0).reshape(S, BL, C)
        out[:, i * BL:(i + 1) * BL, :] = oc
    return out


_CACHE = {}


def run(inputs, trace=False, **spmd_kwargs):
    key = "prog"
    if key not in _CACHE:
        _CACHE[key] = build_program()
    nc = _CACHE[key]
    in_maps = prepare_core_inputs(inputs)
    res = run_bass_kernel_spmd(nc, in_maps, list(range(N_CORES)), trace=trace,
                               **spmd_kwargs)
    return gather_output(res.results), res


def kernel(**inputs):
    out, _ = run(inputs, trace=False)
    return out


# revision 55
# speedup vs baseline: 1.0130x; 1.0130x over previous
"""Trainium2 Bass kernel for a Bayesian MLP (local reparameterization trick).

Reference computation (per sample s of S=10):
    h1 = leaky_relu(x @ W1m + sqrt(x^2 @ W1v + 1e-12) * eps1_s)         [B, 512]
    h2 = leaky_relu(h1a @ W2m + sqrt(h1a^2 @ W2v + 1e-12) * eps2_s)     (h1a = [h1, 1])
    h3 = leaky_relu(h2a @ W3m + sqrt(h2a^2 @ W3v + 1e-12) * eps3_s)
    out = log_softmax(h3a @ W4m + sqrt(h3a^2 @ W4v + 1e-12) * eps4_s)   [B, 10]

Distribution: data-parallel over the batch axis, B=2048 -> 8 cores x 256 rows.
Small variational parameters replicated on every core.

v2 design notes:
  * activations [feature on partitions, (pair, sample, batch) free]; all ten
    samples live in one free axis of 5*512 = 2560 per feature block
  * mean matmuls in bf16 (stationary reused across the 5 sample-pairs)
  * variance matmuls in fp8 e4m3 with DoubleRow (K=256 per pass):
    hq = 8*h^2 (fp8), wv' = 256*v (fp8), descaled inside the ACT sqrt
  * elementwise work spread over ACT/DVE/Pool; big [128, 2560] ops where
    PSUM granularity allows
  * log-softmax phase at the end (single activation-table switch); the
    mean bias of layer 4 rides the Exp bias and a [2,C] stationary trick
"""

import sys
import os

for _p in ("/opt/trn_rl_repo",):
    if _p not in sys.path and os.path.isdir(_p):
        sys.path.insert(0, _p)

import numpy as np
import ml_dtypes

import concourse.bass as bass
import concourse.bacc as bacc
import concourse.mybir as mybir
from concourse import tile
from concourse.bass_utils import run_bass_kernel_spmd

F32 = mybir.dt.float32
F32R = mybir.dt.float32r
BF16 = mybir.dt.bfloat16
FP8 = mybir.dt.float8e4
AF = mybir.ActivationFunctionType
ALU = mybir.AluOpType
DR = mybir.MatmulPerfMode.DoubleRow


def _register_prelu_add():
    """Fused u = in0 + in1 + s0; out = max(0.01*u, u) as ONE DVE op.

    Replaces the separate tensor add and prelu passes of the local
    reparameterization chain (in1 may live in PSUM)."""
    import concourse.dve_ops as D
    from concourse.dve_spec import Spec, Src0, Src1, C0, C2, maxx, lower
    from concourse.dve_uop import DveOpSpec

    name = "PRELU_ADD_ANT"
    if name in D._SUB_OPCODE_FOR_NAME:
        for o in D.OPS:
            if o.name == name:
                return o
    _b = Src0 + Src1 + C0
    spec = Spec(
        body=maxx(_b * C2, _b),
        reference=lambda in0, in1, s0, s1, imm2: np.maximum(
            (in0.astype(np.float32) + in1 + s0) * imm2,
            in0.astype(np.float32) + in1 + s0),
    )
    opcode = D._CUSTOM_DVE_ROW_BASE + len(D.OPS)
    assert opcode < 0x20
    shas = {}
    for ver in ("v3", "v4"):
        uops = lower(spec, ver=ver)
        shas[ver] = DveOpSpec(name=name, opcode=opcode, uops=uops,
                              rd1_en=True).sha(ver)
    op = D.DveOp(name, spec, subdim=False, uops_sha=shas)
    D.OPS.append(op)
    D._SUB_OPCODE_FOR_NAME[name] = opcode
    return op


PRELU_ADD = _register_prelu_add()

B, D_IN, H, C, S = 2048, 784, 512, 10, 10
N_CORES = 8
BL = B // N_CORES            # 256 rows per core
NP = S // 2                  # 5 sample-pairs
FD = 2 * BL                  # 512 free per pair
FDA = NP * FD                # 2560 free, all pairs
K1 = 7                       # 784 -> 7 chunks of 112... no: 896/128
K1P = 8                      # padded to 8 for fp8 DoubleRow pairing
KH = 4                       # 512/128
FO = 4
SQ_SCALE = 2.0               # hq = SQ_SCALE * h^2 (compile-time, ACT Square)
X2_SCALE = 4.0               # x2q = X2_SCALE * x^2
# The fp8 variance-weight scale WV is chosen at RUNTIME per layer (so that
# constant-v layers land exactly on the fp8 grid); the descale is folded into
# the host-side eps tensors and sqrt-bias APs, so the device sqrt has scale=1.

PAIR_GROUPS = ((0, 1, 2), (3, 4))


def build_program(dbg=False):
    nc = bacc.Bacc("TRN2", target_bir_lowering=False, debug=False)

    # ---- DRAM I/O (per core) ----
    xT_d = nc.dram_tensor("xT", [K1, 128, BL], BF16, kind="ExternalInput")
    x2q_d = nc.dram_tensor("x2q", [128, K1P, BL], FP8, kind="ExternalInput")
    w1m_d = nc.dram_tensor("w1m", [K1, 128, H], BF16, kind="ExternalInput")
    w1vq_d = nc.dram_tensor("w1vq", [128, K1P, H], FP8, kind="ExternalInput")
    w2m_d = nc.dram_tensor("w2m", [128, KH, H], BF16, kind="ExternalInput")
    w2vq_d = nc.dram_tensor("w2vq", [128, KH, H], FP8, kind="ExternalInput")
    w3m_d = nc.dram_tensor("w3m", [128, KH, H], BF16, kind="ExternalInput")
    w3vq_d = nc.dram_tensor("w3vq", [128, KH, H], FP8, kind="ExternalInput")
    w4m_d = nc.dram_tensor("w4m", [128, KH, C], BF16, kind="ExternalInput")
    w4vq_d = nc.dram_tensor("w4vq", [128, KH, C], FP8, kind="ExternalInput")
    b2mP_d = nc.dram_tensor("b2mP", [128, FO], F32, kind="ExternalInput")
    b3mP_d = nc.dram_tensor("b3mP", [128, FO], F32, kind="ExternalInput")
    b2v_d = nc.dram_tensor("b2v", [128, FO], F32, kind="ExternalInput")
    b3v_d = nc.dram_tensor("b3v", [128, FO], F32, kind="ExternalInput")
    b4v_d = nc.dram_tensor("b4v", [C, 1], F32, kind="ExternalInput")
    b4m_d = nc.dram_tensor("b4m", [C, 1], F32, kind="ExternalInput")
    bneg_d = nc.dram_tensor("bneg", [2, C], F32R, kind="ExternalInput")
    e1_d = nc.dram_tensor("e1", [FO, 128, FDA], BF16, kind="ExternalInput")
    e2_d = nc.dram_tensor("e2", [FO, 128, FDA], BF16, kind="ExternalInput")
    e3_d = nc.dram_tensor("e3", [FO, 128, FDA], BF16, kind="ExternalInput")
    e4_d = nc.dram_tensor("e4", [C, FDA], BF16, kind="ExternalInput")
    ones_row_d = nc.dram_tensor("ones_row_in", [1, FD], F32R, kind="ExternalInput")
    ones10_d = nc.dram_tensor("ones10_in", [C, 1], F32R, kind="ExternalInput")
    out_d = nc.dram_tensor("out", [C, FDA], F32, kind="ExternalOutput")
    if dbg:
        dbg_sig1 = nc.dram_tensor("dbg_sig1", [128, FO * FD], F32,
                                  kind="ExternalOutput")
        dbg_mu1 = nc.dram_tensor("dbg_mu1", [128, FO * FD], BF16,
                                 kind="ExternalOutput")
        dbg_h1 = nc.dram_tensor("dbg_h1", [FO, 128, FDA], BF16,
                                kind="ExternalOutput")
        dbg_hq1 = nc.dram_tensor("dbg_hq1", [2, 128, 2 * FDA], FP8,
                                 kind="ExternalOutput")
        dbg_h2 = nc.dram_tensor("dbg_h2", [FO, 128, FDA], BF16,
                                kind="ExternalOutput")
        dbg_u4 = nc.dram_tensor("dbg_u4", [C, FDA], F32,
                                kind="ExternalOutput")

    def mm(out_ap, lhsT_ap, rhs_ap, start, stop, perf_mode=None):
        nc.tensor.matmul(out_ap, lhsT_ap, rhs_ap, start=start, stop=stop,
                         perf_mode=perf_mode)

    with tile.TileContext(nc) as tc:
        with (
            tc.tile_pool(name="wp", bufs=1) as wp,
            tc.tile_pool(name="sp", bufs=1) as sp,
            tc.tile_pool(name="hp", bufs=1) as hp,
            tc.tile_pool(name="ep", bufs=1) as ep,
            tc.tile_pool(name="tp", bufs=1) as tp,
        ):
            # ---- persistent weights ----
            w1m_t = [wp.tile([128, H], BF16, tag=f"w1m{k}", name=f"w1m{k}")
                     for k in range(K1)]
            w1vq_t = wp.tile([128, K1P, H], FP8, tag="w1vq", name="w1vq")
            xT_t = [wp.tile([128, BL], BF16, tag=f"xT{k}", name=f"xT{k}")
                    for k in range(K1)]
            x2q_t = wp.tile([128, K1P, BL], FP8, tag="x2q", name="x2q")
            w2m_t = wp.tile([128, KH, H], BF16, tag="w2m", name="w2m")
            w2vq_t = wp.tile([128, KH, H], FP8, tag="w2vq", name="w2vq")
            w3m_t = wp.tile([128, KH, H], BF16, tag="w3m", name="w3m")
            w3vq_t = wp.tile([128, KH, H], FP8, tag="w3vq", name="w3vq")
            w4m_t = wp.tile([128, KH, C], BF16, tag="w4m", name="w4m")
            w4vq_t = wp.tile([128, KH, C], FP8, tag="w4vq", name="w4vq")
            b2mP_t = wp.tile([128, FO], F32, tag="b2mP", name="b2mP")
            b3mP_t = wp.tile([128, FO], F32, tag="b3mP", name="b3mP")
            b2v_t = wp.tile([128, FO], F32, tag="b2v", name="b2v")
            b3v_t = wp.tile([128, FO], F32, tag="b3v", name="b3v")
            b4v_t = wp.tile([C, 1], F32, tag="b4v", name="b4v")
            b4m_t = wp.tile([C, 1], F32, tag="b4m", name="b4m")
            bneg_t = wp.tile([2, C], F32R, tag="bneg", name="bneg")
            ones10 = wp.tile([C, 1], F32R, tag="ones10", name="ones10")
            eps12_t = wp.tile([128, 1], F32, tag="eps12", name="eps12")
            nc.vector.memset(eps12_t[:], 1e-12)

            # ---- persistent activations ----
            # si-duplicated layer-1 stats (so L1 ops need no broadcast reads)
            mu1e_t = sp.tile([128, FO * FD], BF16, tag="mu1e", name="mu1e")
            sig1e_t = sp.tile([128, FO * FD], F32, tag="sig1e", name="sig1e")
            u4_all = sp.tile([C, FDA], F32, tag="u4", name="u4")
            e4_t = sp.tile([C, FDA], BF16, tag="e4", name="e4")
            lsem_t = [sp.tile([2, FD], F32R, tag=f"lsem{i}", name=f"lsem{i}")
                      for i in range(2)]

            # ---- weight DMAs ----
            for k in range(K1):
                nc.sync.dma_start(w1m_t[k][:], w1m_d[k])
                nc.sync.dma_start(xT_t[k][:], xT_d[k])
            nc.sync.dma_start(w1vq_t[:], w1vq_d[:])
            nc.sync.dma_start(x2q_t[:], x2q_d[:])
            nc.sync.dma_start(w2m_t[:], w2m_d[:])
            nc.sync.dma_start(w2vq_t[:], w2vq_d[:])
            nc.sync.dma_start(w3m_t[:], w3m_d[:])
            nc.sync.dma_start(w3vq_t[:], w3vq_d[:])
            nc.sync.dma_start(w4m_t[:], w4m_d[:])
            nc.sync.dma_start(w4vq_t[:], w4vq_d[:])
            nc.sync.dma_start(b2mP_t[:], b2mP_d[:])
            nc.sync.dma_start(b3mP_t[:], b3mP_d[:])
            nc.sync.dma_start(b2v_t[:], b2v_d[:])
            nc.sync.dma_start(b3v_t[:], b3v_d[:])
            nc.sync.dma_start(b4v_t[:], b4v_d[:])
            nc.sync.dma_start(b4m_t[:], b4m_d[:])
            nc.sync.dma_start(bneg_t[:], bneg_d[:])
            nc.sync.dma_start(ones10[:], ones10_d[:])
            nc.sync.dma_start(e4_t[:], e4_d[:])
            for i in range(2):
                nc.sync.dma_start(lsem_t[i][1:2, :], ones_row_d[:])

            # eps tiles: tag per fo, double-buffered across layers
            def load_eps(e_d):
                ts = []
                for fo in range(FO):
                    t = ep.tile([128, FDA], BF16, tag=f"e{fo}", name=f"e{fo}",
                                bufs=2)
                    nc.sync.dma_start(t[:], e_d[fo])
                    ts.append(t)
                return ts

            e1_t = load_eps(e1_d)

            # h/hq tiles: tag per fo / kp, double-buffered across layers
            def h_tiles():
                return [hp.tile([128, FDA], BF16, tag=f"h{fo}", name=f"h{fo}",
                                bufs=2) for fo in range(FO)]

            def hq_tiles():
                return [hp.tile([128, 2, FDA], FP8, tag=f"hq{kp}",
                                name=f"hq{kp}", bufs=2) for kp in range(2)]

            with tc.tile_pool(name="ps", bufs=1, space="PSUM") as ps:
                def mu_ps_tile():
                    return ps.tile([128, FD], F32, tag="mu", name="mu", bufs=4)

                def var_ps_tile():
                    return ps.tile([128, 2 * FD], F32, tag="var2", name="var2",
                                   bufs=2)

                # ---------- Phase A + L1, interleaved per feature block -------
                # L1(fo)'s elementwise rides behind phase A's matmuls for the
                # later feature blocks, so the PE-idle L1 zone shrinks.
                h1_t = h_tiles()
                hq1_t = hq_tiles()
                for fo in range(FO):
                    fs = slice(fo * 128, (fo + 1) * 128)
                    es = slice(fo * FD, (fo + 1) * FD)
                    mu_ps = mu_ps_tile()
                    for k in range(K1):
                        mm(mu_ps[:, 0:BL], w1m_t[k][:, fs], xT_t[k][:],
                           start=(k == 0), stop=(k == K1 - 1))
                    var_ps = var_ps_tile()
                    for kp in range(K1P // 2):
                        mm(var_ps[:, 0:BL], w1vq_t[:, 2 * kp:2 * kp + 2, fs],
                           x2q_t[:, 2 * kp:2 * kp + 2, :],
                           start=(kp == 0), stop=(kp == K1P // 2 - 1),
                           perf_mode=DR)
                    for si in range(2):
                        ss = slice(fo * FD + si * BL, fo * FD + (si + 1) * BL)
                        nc.scalar.activation(sig1e_t[:, ss], var_ps[:, 0:BL],
                                             AF.Sqrt, bias=eps12_t[:])
                        nc.vector.tensor_copy(mu1e_t[:, ss], mu_ps[:, 0:BL])
                    t_l = {}
                    for p in range(NP):
                        sl = slice(p * FD, (p + 1) * FD)
                        t_l[p] = tp.tile([128, FD], BF16, tag="t", name="t",
                                         bufs=3)
                        if p < 3:
                            nc.gpsimd.tensor_tensor(t_l[p][:], sig1e_t[:, es],
                                                    e1_t[fo][:, sl], ALU.mult)
                        else:
                            nc.vector.tensor_tensor(t_l[p][:], sig1e_t[:, es],
                                                    e1_t[fo][:, sl], ALU.mult)
                    for p in range(NP):
                        sl = slice(p * FD, (p + 1) * FD)
                        nc.vector._custom_dve(
                            PRELU_ADD, out=h1_t[fo][:, sl], in0=t_l[p][:],
                            in1=mu1e_t[:, es], s0=0.0, imm2=0.01)
                    nc.scalar.activation(hq1_t[fo // 2][:, fo % 2, :],
                                         h1_t[fo][:], AF.Square, bias=0.0,
                                         scale=float(SQ_SCALE ** 0.5))

                e2_t = load_eps(e2_d)

                # ---------- hidden layers ----------
                VAR_PAIRS = ((0, 1), (2, 3), (4,))

                def hidden_layer(h_in, hq_in, e_t, wm_t, wvq_t, bmP_t, bv_t):
                    h_o = h_tiles()
                    hq_o = hq_tiles()
                    for fo in range(FO):
                        fs = slice(fo * 128, (fo + 1) * 128)
                        sig_t = tp.tile([128, FDA], F32, tag="sigf",
                                        name="sig", bufs=2)
                        # var matmuls first: two pairs share one 2-bank tile
                        for vg in VAR_PAIRS:
                            vt = ps.tile([128, 2 * FD], F32, tag="var2",
                                         name="var2", bufs=2)
                            for kp in range(2):
                                for j, p in enumerate(vg):
                                    mm(vt[:, j * FD:(j + 1) * FD],
                                       wvq_t[:, 2 * kp:2 * kp + 2, fs],
                                       hq_in[kp][:, :, p * FD:(p + 1) * FD],
                                       start=(kp == 0), stop=(kp == 1),
                                       perf_mode=DR)
                            w = len(vg) * FD
                            nc.scalar.activation(
                                sig_t[:, vg[0] * FD:vg[0] * FD + w],
                                vt[:, 0:w], AF.Sqrt, bias=bv_t[:, fo:fo + 1])
                        # mean matmuls p-outer (ldw-opt is off anyway), then
                        # the fused mult / prelu-add chain per pair
                        for p in range(NP):
                            sl = slice(p * FD, (p + 1) * FD)
                            mu_p = mu_ps_tile()
                            for k in range(KH):
                                mm(mu_p[:], wm_t[:, k, fs],
                                   h_in[k][:, sl],
                                   start=(k == 0), stop=(k == KH - 1))
                            t_p = tp.tile([128, FD], BF16, tag="t", name="t",
                                          bufs=3)
                            nc.gpsimd.tensor_tensor(
                                t_p[:], sig_t[:, sl], e_t[fo][:, sl], ALU.mult)
                            nc.vector._custom_dve(
                                PRELU_ADD, out=h_o[fo][:, sl], in0=t_p[:],
                                in1=mu_p[:], s0=bmP_t[:, fo:fo + 1],
                                imm2=0.01)
                        nc.scalar.activation(
                            hq_o[fo // 2][:, fo % 2, :], h_o[fo][:],
                            AF.Square, bias=0.0,
                            scale=float(SQ_SCALE ** 0.5))
                    return h_o, hq_o

                if dbg:
                    nc.sync.dma_start(dbg_sig1[:], sig1e_t[:])
                    nc.sync.dma_start(dbg_mu1[:], mu1e_t[:])
                    for fo in range(FO):
                        nc.sync.dma_start(dbg_h1[fo], h1_t[fo][:])
                    for kp in range(2):
                        nc.sync.dma_start(
                            dbg_hq1[kp],
                            hq1_t[kp][:].rearrange("p a b -> p (a b)"))

                h2_t, hq2_t = hidden_layer(h1_t, hq1_t, e2_t, w2m_t, w2vq_t,
                                           b2mP_t, b2v_t)
                if dbg:
                    for fo in range(FO):
                        nc.sync.dma_start(dbg_h2[fo], h2_t[fo][:])
                e3_t = load_eps(e3_d)
                h3_t, hq3_t = hidden_layer(h2_t, hq2_t, e3_t, w3m_t, w3vq_t,
                                           b3mP_t, b3v_t)

                # ---------- L4 ----------
                for p in range(NP):
                    sl = slice(p * FD, (p + 1) * FD)
                    var_ps = var_ps_tile()
                    for k in range(KH):
                        mm(var_ps[0:C, 0:FD], w4vq_t[:, k, :],
                           hq3_t[k // 2][:, k % 2, sl],
                           start=(k == 0), stop=(k == KH - 1))
                    mu_ps = mu_ps_tile()
                    for k in range(KH):
                        mm(mu_ps[0:C, :], w4m_t[:, k, :], h3_t[k][:, sl],
                           start=(k == 0), stop=(k == KH - 1))
                    sig4_t = tp.tile([C, FD], BF16, tag="sig4", name="sig4",
                                     bufs=2)
                    nc.scalar.activation(sig4_t[:], var_ps[0:C, 0:FD], AF.Sqrt,
                                         bias=b4v_t[:])
                    t4_t = tp.tile([C, FD], BF16, tag="t4", name="t4", bufs=2)
                    nc.vector.tensor_tensor(t4_t[:], sig4_t[:], e4_t[:, sl],
                                            ALU.mult)
                    nc.vector.tensor_tensor(u4_all[:, sl], t4_t[:],
                                            mu_ps[0:C, :], ALU.add)

            if dbg:
                nc.sync.dma_start(dbg_u4[:], u4_all[:])

            # ---------- Phase C: log-softmax (exp/ln table) ----------
            # all Exps first, then all Lns -> exactly two ACT table loads
            with tc.tile_pool(name="psC", bufs=1, space="PSUM") as psC:
                ets, sps = [], []
                for p in range(NP):
                    sl = slice(p * FD, (p + 1) * FD)
                    et = tp.tile([C, FD], F32R, tag="et", name="et", bufs=3)
                    nc.scalar.activation(et[:], u4_all[:, sl], AF.Exp,
                                         bias=b4m_t[:])
                    ets.append(et)
                for p in range(NP):
                    s_ps = psC.tile([1, FD], F32, tag="s", name="s", bufs=5)
                    mm(s_ps[:], ones10[:], ets[p][:], start=True, stop=True)
                    sps.append(s_ps)
                for p in range(NP):
                    sl = slice(p * FD, (p + 1) * FD)
                    lsem = lsem_t[p % 2]
                    nc.scalar.activation(lsem[0:1, :], sps[p][:], AF.Ln,
                                         bias=0.0)
                    lseb_ps = psC.tile([C, FD], F32, tag="lseb", name="lseb",
                                       bufs=2)
                    mm(lseb_ps[:], bneg_t[:], lsem[:], start=True, stop=True)
                    o_t = tp.tile([C, FD], F32, tag="o", name="o", bufs=2)
                    nc.vector.tensor_tensor(o_t[:], u4_all[:, sl], lseb_ps[:],
                                            ALU.subtract)
                    nc.sync.dma_start(out_d[:, sl], o_t[:])

    nc.compile()
    return nc


def _pick_wv(v, base):
    """Scale c ~ base such that typical v*c lands exactly on the fp8 grid.
    For constant-v layers this zeroes the systematic quantization bias."""
    f = np.float32
    pos = v[v > 0]
    if pos.size == 0:
        return f(base)
    vm = f(np.median(pos))
    t = vm * f(base)
    q = f(np.asarray(t, ml_dtypes.float8_e4m3fn).astype(np.float32))
    if q <= 0:
        return f(base)
    return f(q / vm)


def prepare_core_inputs(inputs):
    f = np.float32
    bf = ml_dtypes.bfloat16
    f8 = ml_dtypes.float8_e4m3fn
    x = np.asarray(inputs["inputs"], dtype=f)

    w1m_full = np.asarray(inputs["a1_mean"], f)
    w1m = np.zeros((K1 * 128, H), f)
    w1m[:D_IN] = w1m_full
    w1m = w1m.reshape(K1, 128, H).astype(bf)
    s1 = np.asarray(inputs["a1_dropout"], f) * np.asarray(inputs["a1_scale"], f)
    v1f = (s1 * s1).astype(f)
    wv1 = _pick_wv(v1f, 64.0)
    c1 = X2_SCALE * wv1                      # var_ps = c1 * var
    v1 = np.zeros((K1P * 128, H), f)
    v1[:D_IN] = v1f * wv1
    w1vq = np.ascontiguousarray(
        v1.reshape(K1P, 128, H).transpose(1, 0, 2)).astype(f8)

    def hidden_w(mean, scale, dropout):
        m = np.asarray(mean, f)
        sc = (np.asarray(dropout, f) * np.asarray(scale, f)).astype(f)
        v = sc * sc
        wv = _pick_wv(v[:H], 128.0)
        c = np.float32(SQ_SCALE) * wv        # var_ps = c * var
        wm = np.ascontiguousarray(
            m[:H].reshape(KH, 128, H).transpose(1, 0, 2)).astype(bf)
        wvq = np.ascontiguousarray(
            (v[:H] * wv).reshape(KH, 128, H).transpose(1, 0, 2)).astype(f8)
        bmP = np.ascontiguousarray(m[H].reshape(FO, 128).T).astype(f)
        bv = np.ascontiguousarray(
            ((v[H] + np.float32(1e-12)) * c).reshape(FO, 128).T).astype(f)
        return wm, wvq, bmP, bv, c

    w2m, w2vq, b2mP, b2v, c2 = hidden_w(inputs["a2_mean"], inputs["a2_scale"],
                                        inputs["a2_dropout"])
    w3m, w3vq, b3mP, b3v, c3 = hidden_w(inputs["a3_mean"], inputs["a3_scale"],
                                        inputs["a3_dropout"])

    m4 = np.asarray(inputs["a4_mean"], f)
    s4 = np.asarray(inputs["a4_scale"], f)
    v4 = (s4 * s4).astype(f)
    wv4 = _pick_wv(v4[:H], 128.0)
    c4 = np.float32(SQ_SCALE) * wv4
    w4m = np.ascontiguousarray(
        m4[:H].reshape(KH, 128, C).transpose(1, 0, 2)).astype(bf)
    w4vq = np.ascontiguousarray(
        (v4[:H] * wv4).reshape(KH, 128, C).transpose(1, 0, 2)).astype(f8)
    b4m = np.ascontiguousarray(m4[H].reshape(C, 1)).astype(f)
    b4v = np.ascontiguousarray(((v4[H] + np.float32(1e-12)) * c4)
                               .reshape(C, 1)).astype(f)
    bneg = np.empty((2, C), f)
    bneg[0] = 1.0
    bneg[1] = -m4[H]

    shared = dict(w1m=w1m, w1vq=w1vq, w2m=w2m, w2vq=w2vq, w3m=w3m, w3vq=w3vq,
                  w4m=w4m, w4vq=w4vq, b2mP=b2mP, b3mP=b3mP, b2v=b2v, b3v=b3v,
                  b4v=b4v, b4m=b4m, bneg=bneg,
                  ones_row_in=np.ones((1, FD), dtype=f),
                  ones10_in=np.ones((C, 1), dtype=f))

    eps = [np.asarray(inputs[f"eps{i}"], f) for i in (1, 2, 3, 4)]

    es1 = f(1.0 / np.sqrt(c1))
    es2 = f(1.0 / np.sqrt(c2))
    es3 = f(1.0 / np.sqrt(c3))
    es4 = f(1.0 / np.sqrt(c4))

    def eT(e, b0, es):
        # [S, B, H] -> [FO, 128, (p, si, b)], pre-divided by sqrt(var scale)
        ec = e[:, b0:b0 + BL, :] * es                  # [10, BL, 512]
        return np.ascontiguousarray(
            ec.reshape(NP, 2, BL, FO, 128).transpose(3, 4, 0, 1, 2)
            .reshape(FO, 128, FDA)).astype(bf)

    def e4T(e, b0, es):
        ec = e[:, b0:b0 + BL, :] * es                  # [10, BL, C]
        return np.ascontiguousarray(
            ec.reshape(NP, 2, BL, C).transpose(3, 0, 1, 2)
            .reshape(C, FDA)).astype(bf)

    in_maps = []
    for i in range(N_CORES):
        b0 = i * BL
        xT = np.zeros((K1 * 128, BL), dtype=f)
        xT[:D_IN] = x[b0:b0 + BL].T
        x2 = np.zeros((K1P * 128, BL), dtype=f)
        x2[:D_IN] = (x[b0:b0 + BL].T ** 2) * X2_SCALE
        m = dict(shared)
        m["xT"] = np.ascontiguousarray(xT.reshape(K1, 128, BL)).astype(bf)
        m["x2q"] = np.ascontiguousarray(
            x2.reshape(K1P, 128, BL).transpose(1, 0, 2)).astype(f8)
        m["e1"] = eT(eps[0], b0, es1)
        m["e2"] = eT(eps[1], b0, es2)
        m["e3"] = eT(eps[2], b0, es3)
        m["e4"] = e4T(eps[3], b0, es4)
        in_maps.append(m)
    return in_maps


def gather_output(results):
    out = np.empty((S, B, C), dtype=np.float32)
    for i, r in enumerate(results):
        oc = np.asarray(r["out"])  # [C, (p, si, b)]
        oc = oc.reshape(C, NP, 2, BL).transpose(1, 2, 3, 0).reshape(S, BL, C)
        out[:, i * BL:(i + 1) * BL, :] = oc
    return out


_CACHE = {}


def run(inputs, trace=False, **spmd_kwargs):
    key = "prog"
    if key not in _CACHE:
        _CACHE[key] = build_program()
    nc = _CACHE[key]
    in_maps = prepare_core_inputs(inputs)
    res = run_bass_kernel_spmd(nc, in_maps, list(range(N_CORES)), trace=trace,
                               **spmd_kwargs)
    return gather_output(res.results), res


def kernel(**inputs):
    out, _ = run(inputs, trace=False)
    return out


# revision 57
# speedup vs baseline: 1.0500x; 1.0365x over previous
"""Trainium2 Bass kernel for a Bayesian MLP (local reparameterization trick).

Reference computation (per sample s of S=10):
    h1 = leaky_relu(x @ W1m + sqrt(x^2 @ W1v + 1e-12) * eps1_s)         [B, 512]
    h2 = leaky_relu(h1a @ W2m + sqrt(h1a^2 @ W2v + 1e-12) * eps2_s)     (h1a = [h1, 1])
    h3 = leaky_relu(h2a @ W3m + sqrt(h2a^2 @ W3v + 1e-12) * eps3_s)
    out = log_softmax(h3a @ W4m + sqrt(h3a^2 @ W4v + 1e-12) * eps4_s)   [B, 10]

Distribution: data-parallel over the batch axis, B=2048 -> 8 cores x 256 rows.
Small variational parameters replicated on every core.

v2 design notes:
  * activations [feature on partitions, (pair, sample, batch) free]; all ten
    samples live in one free axis of 5*512 = 2560 per feature block
  * mean matmuls in bf16 (stationary reused across the 5 sample-pairs)
  * variance matmuls in fp8 e4m3 with DoubleRow (K=256 per pass):
    hq = 8*h^2 (fp8), wv' = 256*v (fp8), descaled inside the ACT sqrt
  * elementwise work spread over ACT/DVE/Pool; big [128, 2560] ops where
    PSUM granularity allows
  * log-softmax phase at the end (single activation-table switch); the
    mean bias of layer 4 rides the Exp bias and a [2,C] stationary trick
"""

import sys
import os

for _p in ("/opt/trn_rl_repo",):
    if _p not in sys.path and os.path.isdir(_p):
        sys.path.insert(0, _p)

import numpy as np
import ml_dtypes

import concourse.bass as bass
import concourse.bacc as bacc
import concourse.mybir as mybir
from concourse import tile
from concourse.bass_utils import run_bass_kernel_spmd

F32 = mybir.dt.float32
F32R = mybir.dt.float32r
BF16 = mybir.dt.bfloat16
FP8 = mybir.dt.float8e4
AF = mybir.ActivationFunctionType
ALU = mybir.AluOpType
DR = mybir.MatmulPerfMode.DoubleRow


def _register_prelu_add():
    """Fused u = in0 + in1 + s0; out = max(0.01*u, u) as ONE DVE op.

    Replaces the separate tensor add and prelu passes of the local
    reparameterization chain (in1 may live in PSUM)."""
    import concourse.dve_ops as D
    from concourse.dve_spec import Spec, Src0, Src1, C0, C2, maxx, lower
    from concourse.dve_uop import DveOpSpec

    name = "PRELU_ADD_ANT"
    if name in D._SUB_OPCODE_FOR_NAME:
        for o in D.OPS:
            if o.name == name:
                return o
    _b = Src0 + Src1 + C0
    spec = Spec(
        body=maxx(_b * C2, _b),
        reference=lambda in0, in1, s0, s1, imm2: np.maximum(
            (in0.astype(np.float32) + in1 + s0) * imm2,
            in0.astype(np.float32) + in1 + s0),
    )
    opcode = D._CUSTOM_DVE_ROW_BASE + len(D.OPS)
    assert opcode < 0x20
    shas = {}
    for ver in ("v3", "v4"):
        uops = lower(spec, ver=ver)
        shas[ver] = DveOpSpec(name=name, opcode=opcode, uops=uops,
                              rd1_en=True).sha(ver)
    op = D.DveOp(name, spec, subdim=False, uops_sha=shas)
    D.OPS.append(op)
    D._SUB_OPCODE_FOR_NAME[name] = opcode
    return op


PRELU_ADD = _register_prelu_add()

B, D_IN, H, C, S = 2048, 784, 512, 10, 10
N_CORES = 8
BL = B // N_CORES            # 256 rows per core
NP = S // 2                  # 5 sample-pairs
FD = 2 * BL                  # 512 free per pair
FDA = NP * FD                # 2560 free, all pairs
K1 = 7                       # 784 -> 7 chunks of 112... no: 896/128
K1P = 8                      # padded to 8 for fp8 DoubleRow pairing
KH = 4                       # 512/128
FO = 4
SQ_SCALE = 2.0               # hq = SQ_SCALE * h^2 (compile-time, ACT Square)
X2_SCALE = 4.0               # x2q = X2_SCALE * x^2
# The fp8 variance-weight scale WV is chosen at RUNTIME per layer (so that
# constant-v layers land exactly on the fp8 grid); the descale is folded into
# the host-side eps tensors and sqrt-bias APs, so the device sqrt has scale=1.

PAIR_GROUPS = ((0, 1, 2), (3, 4))


def build_program(dbg=False):
    nc = bacc.Bacc("TRN2", target_bir_lowering=False, debug=False)

    # ---- DRAM I/O (per core) ----
    xT_d = nc.dram_tensor("xT", [K1, 128, BL], BF16, kind="ExternalInput")
    x2q_d = nc.dram_tensor("x2q", [128, K1P, BL], FP8, kind="ExternalInput")
    w1m_d = nc.dram_tensor("w1m", [K1, 128, H], BF16, kind="ExternalInput")
    w1vq_d = nc.dram_tensor("w1vq", [128, K1P, H], FP8, kind="ExternalInput")
    w2m_d = nc.dram_tensor("w2m", [128, KH, H], BF16, kind="ExternalInput")
    w2vq_d = nc.dram_tensor("w2vq", [128, KH, H], FP8, kind="ExternalInput")
    w3m_d = nc.dram_tensor("w3m", [128, KH, H], BF16, kind="ExternalInput")
    w3vq_d = nc.dram_tensor("w3vq", [128, KH, H], FP8, kind="ExternalInput")
    w4m_d = nc.dram_tensor("w4m", [128, KH, C], BF16, kind="ExternalInput")
    w4vq_d = nc.dram_tensor("w4vq", [128, KH, 128], FP8, kind="ExternalInput")
    b2mP_d = nc.dram_tensor("b2mP", [128, FO], F32, kind="ExternalInput")
    b3mP_d = nc.dram_tensor("b3mP", [128, FO], F32, kind="ExternalInput")
    b2v_d = nc.dram_tensor("b2v", [128, FO], F32, kind="ExternalInput")
    b3v_d = nc.dram_tensor("b3v", [128, FO], F32, kind="ExternalInput")
    b4v_d = nc.dram_tensor("b4v", [C, 1], F32, kind="ExternalInput")
    b4m_d = nc.dram_tensor("b4m", [C, 1], F32, kind="ExternalInput")
    bneg_d = nc.dram_tensor("bneg", [2, C], F32R, kind="ExternalInput")
    e1_d = nc.dram_tensor("e1", [FO, 128, FDA], BF16, kind="ExternalInput")
    e2_d = nc.dram_tensor("e2", [FO, 128, FDA], BF16, kind="ExternalInput")
    e3_d = nc.dram_tensor("e3", [FO, 128, FDA], BF16, kind="ExternalInput")
    e4_d = nc.dram_tensor("e4", [C, FDA], BF16, kind="ExternalInput")
    ones_row_d = nc.dram_tensor("ones_row_in", [1, FD], F32R, kind="ExternalInput")
    ones10_d = nc.dram_tensor("ones10_in", [C, 1], F32R, kind="ExternalInput")
    out_d = nc.dram_tensor("out", [C, FDA], F32, kind="ExternalOutput")
    if dbg:
        dbg_sig1 = nc.dram_tensor("dbg_sig1", [128, FO * FD], F32,
                                  kind="ExternalOutput")
        dbg_mu1 = nc.dram_tensor("dbg_mu1", [128, FO * FD], BF16,
                                 kind="ExternalOutput")
        dbg_h1 = nc.dram_tensor("dbg_h1", [FO, 128, FDA], BF16,
                                kind="ExternalOutput")
        dbg_hq1 = nc.dram_tensor("dbg_hq1", [2, 128, 2 * FDA], FP8,
                                 kind="ExternalOutput")
        dbg_h2 = nc.dram_tensor("dbg_h2", [FO, 128, FDA], BF16,
                                kind="ExternalOutput")
        dbg_u4 = nc.dram_tensor("dbg_u4", [C, FDA], F32,
                                kind="ExternalOutput")

    def mm(out_ap, lhsT_ap, rhs_ap, start, stop, perf_mode=None):
        nc.tensor.matmul(out_ap, lhsT_ap, rhs_ap, start=start, stop=stop,
                         perf_mode=perf_mode)

    with tile.TileContext(nc) as tc:
        with (
            tc.tile_pool(name="wp", bufs=1) as wp,
            tc.tile_pool(name="sp", bufs=1) as sp,
            tc.tile_pool(name="hp", bufs=1) as hp,
            tc.tile_pool(name="ep", bufs=1) as ep,
            tc.tile_pool(name="tp", bufs=1) as tp,
        ):
            # ---- persistent weights ----
            w1m_t = [wp.tile([128, H], BF16, tag=f"w1m{k}", name=f"w1m{k}")
                     for k in range(K1)]
            w1vq_t = wp.tile([128, K1P, H], FP8, tag="w1vq", name="w1vq")
            xT_t = [wp.tile([128, BL], BF16, tag=f"xT{k}", name=f"xT{k}")
                    for k in range(K1)]
            x2q_t = wp.tile([128, K1P, BL], FP8, tag="x2q", name="x2q")
            w2m_t = wp.tile([128, KH, H], BF16, tag="w2m", name="w2m")
            w2vq_t = wp.tile([128, KH, H], FP8, tag="w2vq", name="w2vq")
            w3m_t = wp.tile([128, KH, H], BF16, tag="w3m", name="w3m")
            w3vq_t = wp.tile([128, KH, H], FP8, tag="w3vq", name="w3vq")
            w4m_t = wp.tile([128, KH, C], BF16, tag="w4m", name="w4m")
            w4vq_t = wp.tile([128, KH, 128], FP8, tag="w4vq", name="w4vq")
            b2mP_t = wp.tile([128, FO], F32, tag="b2mP", name="b2mP")
            b3mP_t = wp.tile([128, FO], F32, tag="b3mP", name="b3mP")
            b2v_t = wp.tile([128, FO], F32, tag="b2v", name="b2v")
            b3v_t = wp.tile([128, FO], F32, tag="b3v", name="b3v")
            b4v_t = wp.tile([C, 1], F32, tag="b4v", name="b4v")
            b4m_t = wp.tile([C, 1], F32, tag="b4m", name="b4m")
            bneg_t = wp.tile([2, C], F32R, tag="bneg", name="bneg")
            ones10 = wp.tile([C, 1], F32R, tag="ones10", name="ones10")
            eps12_t = wp.tile([128, 1], F32, tag="eps12", name="eps12")
            nc.vector.memset(eps12_t[:], 1e-12)

            # ---- persistent activations ----
            # si-duplicated layer-1 stats (so L1 ops need no broadcast reads)
            sig1e_t = sp.tile([128, FO * FD], F32, tag="sig1e", name="sig1e")
            u4_all = sp.tile([C, FDA], F32, tag="u4", name="u4")
            e4_t = sp.tile([C, FDA], BF16, tag="e4", name="e4")
            lsem_t = [sp.tile([2, FD], F32R, tag=f"lsem{i}", name=f"lsem{i}")
                      for i in range(2)]

            # ---- weight DMAs ----
            for k in range(K1):
                nc.sync.dma_start(w1m_t[k][:], w1m_d[k])
                nc.sync.dma_start(xT_t[k][:], xT_d[k])
            nc.sync.dma_start(w1vq_t[:], w1vq_d[:])
            nc.sync.dma_start(x2q_t[:], x2q_d[:])
            nc.sync.dma_start(w2m_t[:], w2m_d[:])
            nc.sync.dma_start(w2vq_t[:], w2vq_d[:])
            nc.sync.dma_start(w3m_t[:], w3m_d[:])
            nc.sync.dma_start(w3vq_t[:], w3vq_d[:])
            nc.sync.dma_start(w4m_t[:], w4m_d[:])
            nc.sync.dma_start(w4vq_t[:], w4vq_d[:])
            nc.sync.dma_start(b2mP_t[:], b2mP_d[:])
            nc.sync.dma_start(b3mP_t[:], b3mP_d[:])
            nc.sync.dma_start(b2v_t[:], b2v_d[:])
            nc.sync.dma_start(b3v_t[:], b3v_d[:])
            nc.sync.dma_start(b4v_t[:], b4v_d[:])
            nc.sync.dma_start(b4m_t[:], b4m_d[:])
            nc.sync.dma_start(bneg_t[:], bneg_d[:])
            nc.sync.dma_start(ones10[:], ones10_d[:])
            nc.sync.dma_start(e4_t[:], e4_d[:])
            for i in range(2):
                nc.sync.dma_start(lsem_t[i][1:2, :], ones_row_d[:])

            # eps tiles: tag per fo, double-buffered across layers
            def load_eps(e_d):
                ts = []
                for fo in range(FO):
                    t = ep.tile([128, FDA], BF16, tag=f"e{fo}", name=f"e{fo}",
                                bufs=2)
                    nc.sync.dma_start(t[:], e_d[fo])
                    ts.append(t)
                return ts

            e1_t = load_eps(e1_d)

            # h/hq tiles: tag per fo / kp, double-buffered across layers
            def h_tiles():
                return [hp.tile([128, FDA], BF16, tag=f"h{fo}", name=f"h{fo}",
                                bufs=2) for fo in range(FO)]

            def hq_tiles():
                return [hp.tile([128, 2, FDA], FP8, tag=f"hq{kp}",
                                name=f"hq{kp}", bufs=2) for kp in range(2)]

            with tc.tile_pool(name="ps", bufs=1, space="PSUM") as ps:
                def mu_ps_tile():
                    return ps.tile([128, FD], F32, tag="mu", name="mu", bufs=4)

                def var_ps_tile():
                    return ps.tile([128, 2 * FD], F32, tag="var2", name="var2",
                                   bufs=2)

                # ---------- Phase A + L1, interleaved per feature block -------
                # L1(fo)'s elementwise rides behind phase A's matmuls for the
                # later feature blocks, so the PE-idle L1 zone shrinks.
                h1_t = h_tiles()
                hq1_t = hq_tiles()
                for fo in range(FO):
                    fs = slice(fo * 128, (fo + 1) * 128)
                    es = slice(fo * FD, (fo + 1) * FD)
                    mu_ps = mu_ps_tile()
                    for k in range(K1):
                        mm(mu_ps[:, 0:BL], w1m_t[k][:, fs], xT_t[k][:],
                           start=(k == 0), stop=(k == K1 - 1))
                    var_ps = var_ps_tile()
                    for kp in range(K1P // 2):
                        mm(var_ps[:, 0:BL], w1vq_t[:, 2 * kp:2 * kp + 2, fs],
                           x2q_t[:, 2 * kp:2 * kp + 2, :],
                           start=(kp == 0), stop=(kp == K1P // 2 - 1),
                           perf_mode=DR)
                    for si in range(2):
                        ss = slice(fo * FD + si * BL, fo * FD + (si + 1) * BL)
                        nc.scalar.activation(sig1e_t[:, ss], var_ps[:, 0:BL],
                                             AF.Sqrt, bias=eps12_t[:])
                    t_l = {}
                    for p in range(NP):
                        sl = slice(p * FD, (p + 1) * FD)
                        t_l[p] = tp.tile([128, FD], BF16, tag="t", name="t",
                                         bufs=3)
                        if p < 3:
                            nc.gpsimd.tensor_tensor(t_l[p][:], sig1e_t[:, es],
                                                    e1_t[fo][:, sl], ALU.mult)
                        else:
                            nc.vector.tensor_tensor(t_l[p][:], sig1e_t[:, es],
                                                    e1_t[fo][:, sl], ALU.mult)
                    mu_b = (mu_ps[:, 0:BL].unsqueeze(1)
                            .broadcast_to((128, 2, BL)))
                    for p in range(NP):
                        sl = slice(p * FD, (p + 1) * FD)
                        nc.vector._custom_dve(
                            PRELU_ADD,
                            out=h1_t[fo][:, sl].rearrange(
                                "q (s n) -> q s n", s=2),
                            in0=mu_b, in1=t_l[p][:], s0=0.0, imm2=0.01)
                    nc.scalar.activation(hq1_t[fo // 2][:, fo % 2, :],
                                         h1_t[fo][:], AF.Square, bias=0.0,
                                         scale=float(SQ_SCALE ** 0.5))

                e2_t = load_eps(e2_d)

                # ---------- hidden layers ----------
                VAR_PAIRS = ((0, 1), (2, 3), (4,))

                def hidden_layer(h_in, hq_in, e_t, wm_t, wvq_t, bmP_t, bv_t):
                    h_o = h_tiles()
                    hq_o = hq_tiles()
                    for fo in range(FO):
                        fs = slice(fo * 128, (fo + 1) * 128)
                        sig_t = tp.tile([128, FDA], F32, tag="sigf",
                                        name="sig", bufs=2)
                        # var matmuls first: two pairs share one 2-bank tile
                        for vg in VAR_PAIRS:
                            vt = ps.tile([128, 2 * FD], F32, tag="var2",
                                         name="var2", bufs=2)
                            for kp in range(2):
                                for j, p in enumerate(vg):
                                    mm(vt[:, j * FD:(j + 1) * FD],
                                       wvq_t[:, 2 * kp:2 * kp + 2, fs],
                                       hq_in[kp][:, :, p * FD:(p + 1) * FD],
                                       start=(kp == 0), stop=(kp == 1),
                                       perf_mode=DR)
                            w = len(vg) * FD
                            nc.scalar.activation(
                                sig_t[:, vg[0] * FD:vg[0] * FD + w],
                                vt[:, 0:w], AF.Sqrt, bias=bv_t[:, fo:fo + 1])
                        # mean matmuls p-outer (ldw-opt is off anyway), then
                        # the fused mult / prelu-add chain per pair
                        for p in range(NP):
                            sl = slice(p * FD, (p + 1) * FD)
                            mu_p = mu_ps_tile()
                            for k in range(KH):
                                mm(mu_p[:], wm_t[:, k, fs],
                                   h_in[k][:, sl],
                                   start=(k == 0), stop=(k == KH - 1))
                            t_p = tp.tile([128, FD], BF16, tag="t", name="t",
                                          bufs=3)
                            nc.gpsimd.tensor_tensor(
                                t_p[:], sig_t[:, sl], e_t[fo][:, sl], ALU.mult)
                            nc.vector._custom_dve(
                                PRELU_ADD, out=h_o[fo][:, sl], in0=t_p[:],
                                in1=mu_p[:], s0=bmP_t[:, fo:fo + 1],
                                imm2=0.01)
                        nc.scalar.activation(
                            hq_o[fo // 2][:, fo % 2, :], h_o[fo][:],
                            AF.Square, bias=0.0,
                            scale=float(SQ_SCALE ** 0.5))
                    return h_o, hq_o

                if dbg:
                    nc.sync.dma_start(dbg_sig1[:], sig1e_t[:])
                    for fo in range(FO):
                        nc.sync.dma_start(dbg_h1[fo], h1_t[fo][:])
                    for kp in range(2):
                        nc.sync.dma_start(
                            dbg_hq1[kp],
                            hq1_t[kp][:].rearrange("p a b -> p (a b)"))

                h2_t, hq2_t = hidden_layer(h1_t, hq1_t, e2_t, w2m_t, w2vq_t,
                                           b2mP_t, b2v_t)
                if dbg:
                    for fo in range(FO):
                        nc.sync.dma_start(dbg_h2[fo], h2_t[fo][:])
                e3_t = load_eps(e3_d)
                h3_t, hq3_t = hidden_layer(h2_t, hq2_t, e3_t, w3m_t, w3vq_t,
                                           b3mP_t, b3v_t)

                # ---------- L4 ----------
                for p in range(NP):
                    sl = slice(p * FD, (p + 1) * FD)
                    var_ps = var_ps_tile()
                    for kp in range(2):
                        mm(var_ps[:, 0:FD], w4vq_t[:, 2 * kp:2 * kp + 2, :],
                           hq3_t[kp][:, :, sl],
                           start=(kp == 0), stop=(kp == 1), perf_mode=DR)
                    mu_ps = mu_ps_tile()
                    for k in range(KH):
                        mm(mu_ps[0:C, :], w4m_t[:, k, :], h3_t[k][:, sl],
                           start=(k == 0), stop=(k == KH - 1))
                    sig4_t = tp.tile([C, FD], BF16, tag="sig4", name="sig4",
                                     bufs=2)
                    nc.scalar.activation(sig4_t[:], var_ps[0:C, 0:FD], AF.Sqrt,
                                         bias=b4v_t[:])
                    t4_t = tp.tile([C, FD], BF16, tag="t4", name="t4", bufs=2)
                    nc.vector.tensor_tensor(t4_t[:], sig4_t[:], e4_t[:, sl],
                                            ALU.mult)
                    nc.vector.tensor_tensor(u4_all[:, sl], t4_t[:],
                                            mu_ps[0:C, :], ALU.add)

            if dbg:
                nc.sync.dma_start(dbg_u4[:], u4_all[:])

            # ---------- Phase C: log-softmax (exp/ln table) ----------
            # all Exps first, then all Lns -> exactly two ACT table loads
            with tc.tile_pool(name="psC", bufs=1, space="PSUM") as psC:
                ets, sps = [], []
                for p in range(NP):
                    sl = slice(p * FD, (p + 1) * FD)
                    et = tp.tile([C, FD], F32R, tag="et", name="et", bufs=3)
                    nc.scalar.activation(et[:], u4_all[:, sl], AF.Exp,
                                         bias=b4m_t[:])
                    ets.append(et)
                for p in range(NP):
                    s_ps = psC.tile([1, FD], F32, tag="s", name="s", bufs=5)
                    mm(s_ps[:], ones10[:], ets[p][:], start=True, stop=True)
                    sps.append(s_ps)
                for p in range(NP):
                    sl = slice(p * FD, (p + 1) * FD)
                    lsem = lsem_t[p % 2]
                    nc.scalar.activation(lsem[0:1, :], sps[p][:], AF.Ln,
                                         bias=0.0)
                    lseb_ps = psC.tile([C, FD], F32, tag="lseb", name="lseb",
                                       bufs=2)
                    mm(lseb_ps[:], bneg_t[:], lsem[:], start=True, stop=True)
                    o_t = tp.tile([C, FD], F32, tag="o", name="o", bufs=2)
                    nc.vector.tensor_tensor(o_t[:], u4_all[:, sl], lseb_ps[:],
                                            ALU.subtract)
                    nc.sync.dma_start(out_d[:, sl], o_t[:])

    nc.compile()
    return nc


def _pick_wv(v, base):
    """Scale c ~ base such that typical v*c lands exactly on the fp8 grid.
    For constant-v layers this zeroes the systematic quantization bias."""
    f = np.float32
    pos = v[v > 0]
    if pos.size == 0:
        return f(base)
    vm = f(np.median(pos))
    t = vm * f(base)
    q = f(np.asarray(t, ml_dtypes.float8_e4m3fn).astype(np.float32))
    if q <= 0:
        return f(base)
    return f(q / vm)


def prepare_core_inputs(inputs):
    f = np.float32
    bf = ml_dtypes.bfloat16
    f8 = ml_dtypes.float8_e4m3fn
    x = np.asarray(inputs["inputs"], dtype=f)

    w1m_full = np.asarray(inputs["a1_mean"], f)
    w1m = np.zeros((K1 * 128, H), f)
    w1m[:D_IN] = w1m_full
    w1m = w1m.reshape(K1, 128, H).astype(bf)
    s1 = np.asarray(inputs["a1_dropout"], f) * np.asarray(inputs["a1_scale"], f)
    v1f = (s1 * s1).astype(f)
    wv1 = _pick_wv(v1f, 64.0)
    c1 = X2_SCALE * wv1                      # var_ps = c1 * var
    v1 = np.zeros((K1P * 128, H), f)
    v1[:D_IN] = v1f * wv1
    w1vq = np.ascontiguousarray(
        v1.reshape(K1P, 128, H).transpose(1, 0, 2)).astype(f8)

    def hidden_w(mean, scale, dropout):
        m = np.asarray(mean, f)
        sc = (np.asarray(dropout, f) * np.asarray(scale, f)).astype(f)
        v = sc * sc
        wv = _pick_wv(v[:H], 128.0)
        c = np.float32(SQ_SCALE) * wv        # var_ps = c * var
        wm = np.ascontiguousarray(
            m[:H].reshape(KH, 128, H).transpose(1, 0, 2)).astype(bf)
        wvq = np.ascontiguousarray(
            (v[:H] * wv).reshape(KH, 128, H).transpose(1, 0, 2)).astype(f8)
        bmP = np.ascontiguousarray(m[H].reshape(FO, 128).T).astype(f)
        bv = np.ascontiguousarray(
            ((v[H] + np.float32(1e-12)) * c).reshape(FO, 128).T).astype(f)
        return wm, wvq, bmP, bv, c

    w2m, w2vq, b2mP, b2v, c2 = hidden_w(inputs["a2_mean"], inputs["a2_scale"],
                                        inputs["a2_dropout"])
    w3m, w3vq, b3mP, b3v, c3 = hidden_w(inputs["a3_mean"], inputs["a3_scale"],
                                        inputs["a3_dropout"])

    m4 = np.asarray(inputs["a4_mean"], f)
    s4 = np.asarray(inputs["a4_scale"], f)
    v4 = (s4 * s4).astype(f)
    wv4 = _pick_wv(v4[:H], 128.0)
    c4 = np.float32(SQ_SCALE) * wv4
    w4m = np.ascontiguousarray(
        m4[:H].reshape(KH, 128, C).transpose(1, 0, 2)).astype(bf)
    w4vq_s = (v4[:H] * wv4).reshape(KH, 128, C)
    w4vq_p = np.zeros((KH, 128, 128), np.float32)
    w4vq_p[:, :, :C] = w4vq_s
    w4vq = np.ascontiguousarray(w4vq_p.transpose(1, 0, 2)).astype(f8)
    b4m = np.ascontiguousarray(m4[H].reshape(C, 1)).astype(f)
    b4v = np.ascontiguousarray(((v4[H] + np.float32(1e-12)) * c4)
                               .reshape(C, 1)).astype(f)
    bneg = np.empty((2, C), f)
    bneg[0] = 1.0
    bneg[1] = -m4[H]

    shared = dict(w1m=w1m, w1vq=w1vq, w2m=w2m, w2vq=w2vq, w3m=w3m, w3vq=w3vq,
                  w4m=w4m, w4vq=w4vq, b2mP=b2mP, b3mP=b3mP, b2v=b2v, b3v=b3v,
                  b4v=b4v, b4m=b4m, bneg=bneg,
                  ones_row_in=np.ones((1, FD), dtype=f),
                  ones10_in=np.ones((C, 1), dtype=f))

    eps = [np.asarray(inputs[f"eps{i}"], f) for i in (1, 2, 3, 4)]

    es1 = f(1.0 / np.sqrt(c1))
    es2 = f(1.0 / np.sqrt(c2))
    es3 = f(1.0 / np.sqrt(c3))
    es4 = f(1.0 / np.sqrt(c4))

    def eT(e, b0, es):
        # [S, B, H] -> [FO, 128, (p, si, b)], pre-divided by sqrt(var scale)
        ec = e[:, b0:b0 + BL, :] * es                  # [10, BL, 512]
        return np.ascontiguousarray(
            ec.reshape(NP, 2, BL, FO, 128).transpose(3, 4, 0, 1, 2)
            .reshape(FO, 128, FDA)).astype(bf)

    def e4T(e, b0, es):
        ec = e[:, b0:b0 + BL, :] * es                  # [10, BL, C]
        return np.ascontiguousarray(
            ec.reshape(NP, 2, BL, C).transpose(3, 0, 1, 2)
            .reshape(C, FDA)).astype(bf)

    in_maps = []
    for i in range(N_CORES):
        b0 = i * BL
        xT = np.zeros((K1 * 128, BL), dtype=f)
        xT[:D_IN] = x[b0:b0 + BL].T
        x2 = np.zeros((K1P * 128, BL), dtype=f)
        x2[:D_IN] = (x[b0:b0 + BL].T ** 2) * X2_SCALE
        m = dict(shared)
        m["xT"] = np.ascontiguousarray(xT.reshape(K1, 128, BL)).astype(bf)
        m["x2q"] = np.ascontiguousarray(
            x2.reshape(K1P, 128, BL).transpose(1, 0, 2)).astype(f8)
        m["e1"] = eT(eps[0], b0, es1)
        m["e2"] = eT(eps[1], b0, es2)
        m["e3"] = eT(eps[2], b0, es3)
        m["e4"] = e4T(eps[3], b0, es4)
        in_maps.append(m)
    return in_maps


def gather_output(results):
    out = np.empty((S, B, C), dtype=np.float32)
    for i, r in enumerate(results):
        oc = np.asarray(r["out"])  # [C, (p, si, b)]
        oc = oc.reshape(C, NP, 2, BL).transpose(1, 2, 3, 0).reshape(S, BL, C)
        out[:, i * BL:(i + 1) * BL, :] = oc
    return out


_CACHE = {}


def run(inputs, trace=False, **spmd_kwargs):
    key = "prog"
    if key not in _CACHE:
        _CACHE[key] = build_program()
    nc = _CACHE[key]
    in_maps = prepare_core_inputs(inputs)
    res = run_bass_kernel_spmd(nc, in_maps, list(range(N_CORES)), trace=trace,
                               **spmd_kwargs)
    return gather_output(res.results), res


def kernel(**inputs):
    out, _ = run(inputs, trace=False)
    return out


# revision 58
# speedup vs baseline: 1.0869x; 1.0352x over previous
"""Trainium2 Bass kernel for a Bayesian MLP (local reparameterization trick).

Reference computation (per sample s of S=10):
    h1 = leaky_relu(x @ W1m + sqrt(x^2 @ W1v + 1e-12) * eps1_s)         [B, 512]
    h2 = leaky_relu(h1a @ W2m + sqrt(h1a^2 @ W2v + 1e-12) * eps2_s)     (h1a = [h1, 1])
    h3 = leaky_relu(h2a @ W3m + sqrt(h2a^2 @ W3v + 1e-12) * eps3_s)
    out = log_softmax(h3a @ W4m + sqrt(h3a^2 @ W4v + 1e-12) * eps4_s)   [B, 10]

Distribution: data-parallel over the batch axis, B=2048 -> 8 cores x 256 rows.
Small variational parameters replicated on every core.

v2 design notes:
  * activations [feature on partitions, (pair, sample, batch) free]; all ten
    samples live in one free axis of 5*512 = 2560 per feature block
  * mean matmuls in bf16 (stationary reused across the 5 sample-pairs)
  * variance matmuls in fp8 e4m3 with DoubleRow (K=256 per pass):
    hq = 8*h^2 (fp8), wv' = 256*v (fp8), descaled inside the ACT sqrt
  * elementwise work spread over ACT/DVE/Pool; big [128, 2560] ops where
    PSUM granularity allows
  * log-softmax phase at the end (single activation-table switch); the
    mean bias of layer 4 rides the Exp bias and a [2,C] stationary trick
"""

import sys
import os

for _p in ("/opt/trn_rl_repo",):
    if _p not in sys.path and os.path.isdir(_p):
        sys.path.insert(0, _p)

import numpy as np
import ml_dtypes

import concourse.bass as bass
import concourse.bacc as bacc
import concourse.mybir as mybir
from concourse import tile
from concourse.bass_utils import run_bass_kernel_spmd

F32 = mybir.dt.float32
F32R = mybir.dt.float32r
BF16 = mybir.dt.bfloat16
FP8 = mybir.dt.float8e4
AF = mybir.ActivationFunctionType
ALU = mybir.AluOpType
DR = mybir.MatmulPerfMode.DoubleRow


def _register_prelu_add():
    """Fused u = in0 + in1 + s0; out = max(0.01*u, u) as ONE DVE op.

    Replaces the separate tensor add and prelu passes of the local
    reparameterization chain (in1 may live in PSUM)."""
    import concourse.dve_ops as D
    from concourse.dve_spec import Spec, Src0, Src1, C0, C2, maxx, lower
    from concourse.dve_uop import DveOpSpec

    name = "PRELU_ADD_ANT"
    if name in D._SUB_OPCODE_FOR_NAME:
        for o in D.OPS:
            if o.name == name:
                return o
    _b = Src0 + Src1 + C0
    spec = Spec(
        body=maxx(_b * C2, _b),
        reference=lambda in0, in1, s0, s1, imm2: np.maximum(
            (in0.astype(np.float32) + in1 + s0) * imm2,
            in0.astype(np.float32) + in1 + s0),
    )
    opcode = D._CUSTOM_DVE_ROW_BASE + len(D.OPS)
    assert opcode < 0x20
    shas = {}
    for ver in ("v3", "v4"):
        uops = lower(spec, ver=ver)
        shas[ver] = DveOpSpec(name=name, opcode=opcode, uops=uops,
                              rd1_en=True).sha(ver)
    op = D.DveOp(name, spec, subdim=False, uops_sha=shas)
    D.OPS.append(op)
    D._SUB_OPCODE_FOR_NAME[name] = opcode
    return op


PRELU_ADD = _register_prelu_add()

B, D_IN, H, C, S = 2048, 784, 512, 10, 10
N_CORES = 8
BL = B // N_CORES            # 256 rows per core
NP = S // 2                  # 5 sample-pairs
FD = 2 * BL                  # 512 free per pair
FDA = NP * FD                # 2560 free, all pairs
K1 = 7                       # 784 -> 7 chunks of 112... no: 896/128
K1P = 8                      # padded to 8 for fp8 DoubleRow pairing
KH = 4                       # 512/128
FO = 4
SQ_SCALE = 2.0               # hq = SQ_SCALE * h^2 (compile-time, ACT Square)
X2_SCALE = 4.0               # x2q = X2_SCALE * x^2
# The fp8 variance-weight scale WV is chosen at RUNTIME per layer (so that
# constant-v layers land exactly on the fp8 grid); the descale is folded into
# the host-side eps tensors and sqrt-bias APs, so the device sqrt has scale=1.

PAIR_GROUPS = ((0, 1, 2), (3, 4))


def build_program(dbg=False):
    nc = bacc.Bacc("TRN2", target_bir_lowering=False, debug=False)

    # ---- DRAM I/O (per core) ----
    xT_d = nc.dram_tensor("xT", [K1, 128, BL], BF16, kind="ExternalInput")
    x2q_d = nc.dram_tensor("x2q", [128, K1P, BL], FP8, kind="ExternalInput")
    w1m_d = nc.dram_tensor("w1m", [K1, 128, H], BF16, kind="ExternalInput")
    w1vq_d = nc.dram_tensor("w1vq", [128, K1P, H], FP8, kind="ExternalInput")
    w2m_d = nc.dram_tensor("w2m", [128, KH, H], BF16, kind="ExternalInput")
    w2vq_d = nc.dram_tensor("w2vq", [128, KH, H], FP8, kind="ExternalInput")
    w3m_d = nc.dram_tensor("w3m", [128, KH, H], BF16, kind="ExternalInput")
    w3vq_d = nc.dram_tensor("w3vq", [128, KH, H], FP8, kind="ExternalInput")
    w4m_d = nc.dram_tensor("w4m", [128, KH, C], BF16, kind="ExternalInput")
    w4vq_d = nc.dram_tensor("w4vq", [128, KH, 128], FP8, kind="ExternalInput")
    b2mP_d = nc.dram_tensor("b2mP", [128, FO], F32, kind="ExternalInput")
    b3mP_d = nc.dram_tensor("b3mP", [128, FO], F32, kind="ExternalInput")
    b2v_d = nc.dram_tensor("b2v", [128, FO], F32, kind="ExternalInput")
    b3v_d = nc.dram_tensor("b3v", [128, FO], F32, kind="ExternalInput")
    b4v_d = nc.dram_tensor("b4v", [C, 1], F32, kind="ExternalInput")
    b4m_d = nc.dram_tensor("b4m", [C, 1], F32, kind="ExternalInput")
    bneg_d = nc.dram_tensor("bneg", [2, C], F32R, kind="ExternalInput")
    e1_d = nc.dram_tensor("e1", [FO, 128, FDA], BF16, kind="ExternalInput")
    e2_d = nc.dram_tensor("e2", [FO, 128, FDA], BF16, kind="ExternalInput")
    e3_d = nc.dram_tensor("e3", [FO, 128, FDA], BF16, kind="ExternalInput")
    e4_d = nc.dram_tensor("e4", [C, FDA], BF16, kind="ExternalInput")
    ones_row_d = nc.dram_tensor("ones_row_in", [1, FD], F32R, kind="ExternalInput")
    ones10_d = nc.dram_tensor("ones10_in", [C, 1], F32R, kind="ExternalInput")
    out_d = nc.dram_tensor("out", [C, FDA], F32, kind="ExternalOutput")
    if dbg:
        dbg_sig1 = nc.dram_tensor("dbg_sig1", [128, FO * FD], F32,
                                  kind="ExternalOutput")
        dbg_mu1 = nc.dram_tensor("dbg_mu1", [128, FO * FD], BF16,
                                 kind="ExternalOutput")
        dbg_h1 = nc.dram_tensor("dbg_h1", [FO, 128, FDA], BF16,
                                kind="ExternalOutput")
        dbg_hq1 = nc.dram_tensor("dbg_hq1", [2, 128, 2 * FDA], FP8,
                                 kind="ExternalOutput")
        dbg_h2 = nc.dram_tensor("dbg_h2", [FO, 128, FDA], BF16,
                                kind="ExternalOutput")
        dbg_u4 = nc.dram_tensor("dbg_u4", [C, FDA], F32,
                                kind="ExternalOutput")

    def mm(out_ap, lhsT_ap, rhs_ap, start, stop, perf_mode=None):
        nc.tensor.matmul(out_ap, lhsT_ap, rhs_ap, start=start, stop=stop,
                         perf_mode=perf_mode)

    with tile.TileContext(nc) as tc:
        with (
            tc.tile_pool(name="wp", bufs=1) as wp,
            tc.tile_pool(name="sp", bufs=1) as sp,
            tc.tile_pool(name="hp", bufs=1) as hp,
            tc.tile_pool(name="ep", bufs=1) as ep,
            tc.tile_pool(name="tp", bufs=1) as tp,
        ):
            # ---- persistent weights ----
            w1m_t = [wp.tile([128, H], BF16, tag=f"w1m{k}", name=f"w1m{k}")
                     for k in range(K1)]
            w1vq_t = wp.tile([128, K1P, H], FP8, tag="w1vq", name="w1vq")
            xT_t = [wp.tile([128, BL], BF16, tag=f"xT{k}", name=f"xT{k}")
                    for k in range(K1)]
            x2q_t = wp.tile([128, K1P, BL], FP8, tag="x2q", name="x2q")
            w2m_t = wp.tile([128, KH, H], BF16, tag="w2m", name="w2m")
            w2vq_t = wp.tile([128, KH, H], FP8, tag="w2vq", name="w2vq")
            w3m_t = wp.tile([128, KH, H], BF16, tag="w3m", name="w3m")
            w3vq_t = wp.tile([128, KH, H], FP8, tag="w3vq", name="w3vq")
            w4m_t = wp.tile([128, KH, C], BF16, tag="w4m", name="w4m")
            w4vq_t = wp.tile([128, KH, 128], FP8, tag="w4vq", name="w4vq")
            b2mP_t = wp.tile([128, FO], F32, tag="b2mP", name="b2mP")
            b3mP_t = wp.tile([128, FO], F32, tag="b3mP", name="b3mP")
            b2v_t = wp.tile([128, FO], F32, tag="b2v", name="b2v")
            b3v_t = wp.tile([128, FO], F32, tag="b3v", name="b3v")
            b4v_t = wp.tile([C, 1], F32, tag="b4v", name="b4v")
            b4m_t = wp.tile([C, 1], F32, tag="b4m", name="b4m")
            bneg_t = wp.tile([2, C], F32R, tag="bneg", name="bneg")
            ones10 = wp.tile([C, 1], F32R, tag="ones10", name="ones10")
            eps12_t = wp.tile([128, 1], F32, tag="eps12", name="eps12")
            nc.vector.memset(eps12_t[:], 1e-12)

            # ---- persistent activations ----
            # si-duplicated layer-1 stats (so L1 ops need no broadcast reads)
            sig1e_t = sp.tile([128, FO * FD], F32, tag="sig1e", name="sig1e")
            u4_all = sp.tile([C, FDA], F32, tag="u4", name="u4")
            e4_t = sp.tile([C, FDA], BF16, tag="e4", name="e4")
            lsem_t = [sp.tile([2, FD], F32R, tag=f"lsem{i}", name=f"lsem{i}")
                      for i in range(2)]

            # ---- weight DMAs ----
            for k in range(K1):
                nc.sync.dma_start(w1m_t[k][:], w1m_d[k])
                nc.sync.dma_start(xT_t[k][:], xT_d[k])
            nc.sync.dma_start(w1vq_t[:], w1vq_d[:])
            nc.sync.dma_start(x2q_t[:], x2q_d[:])
            nc.sync.dma_start(w2m_t[:], w2m_d[:])
            nc.sync.dma_start(w2vq_t[:], w2vq_d[:])
            nc.sync.dma_start(w3m_t[:], w3m_d[:])
            nc.sync.dma_start(w3vq_t[:], w3vq_d[:])
            nc.sync.dma_start(w4m_t[:], w4m_d[:])
            nc.sync.dma_start(w4vq_t[:], w4vq_d[:])
            nc.sync.dma_start(b2mP_t[:], b2mP_d[:])
            nc.sync.dma_start(b3mP_t[:], b3mP_d[:])
            nc.sync.dma_start(b2v_t[:], b2v_d[:])
            nc.sync.dma_start(b3v_t[:], b3v_d[:])
            nc.sync.dma_start(b4v_t[:], b4v_d[:])
            nc.sync.dma_start(b4m_t[:], b4m_d[:])
            nc.sync.dma_start(bneg_t[:], bneg_d[:])
            nc.sync.dma_start(ones10[:], ones10_d[:])
            nc.sync.dma_start(e4_t[:], e4_d[:])
            for i in range(2):
                nc.sync.dma_start(lsem_t[i][1:2, :], ones_row_d[:])

            # eps tiles: tag per fo, double-buffered across layers
            def load_eps(e_d):
                ts = []
                for fo in range(FO):
                    t = ep.tile([128, FDA], BF16, tag=f"e{fo}", name=f"e{fo}",
                                bufs=2)
                    nc.sync.dma_start(t[:], e_d[fo])
                    ts.append(t)
                return ts

            e1_t = load_eps(e1_d)

            # h/hq tiles: tag per fo / kp, double-buffered across layers
            def h_tiles():
                return [hp.tile([128, FDA], BF16, tag=f"h{fo}", name=f"h{fo}",
                                bufs=2) for fo in range(FO)]

            def hq_tiles():
                return [hp.tile([128, 2, FDA], FP8, tag=f"hq{kp}",
                                name=f"hq{kp}", bufs=2) for kp in range(2)]

            with tc.tile_pool(name="ps", bufs=1, space="PSUM") as ps:
                def mu_ps_tile():
                    return ps.tile([128, FD], F32, tag="mu", name="mu", bufs=4)

                def var_ps_tile():
                    return ps.tile([128, 2 * FD], F32, tag="var2", name="var2",
                                   bufs=2)

                # ---------- Phase A + L1, interleaved per feature block -------
                # L1(fo)'s elementwise rides behind phase A's matmuls for the
                # later feature blocks, so the PE-idle L1 zone shrinks.
                h1_t = h_tiles()
                hq1_t = hq_tiles()
                for fo in range(FO):
                    fs = slice(fo * 128, (fo + 1) * 128)
                    es = slice(fo * FD, (fo + 1) * FD)
                    mu_ps = mu_ps_tile()
                    for k in range(K1):
                        mm(mu_ps[:, 0:BL], w1m_t[k][:, fs], xT_t[k][:],
                           start=(k == 0), stop=(k == K1 - 1))
                    var_ps = var_ps_tile()
                    for kp in range(K1P // 2):
                        mm(var_ps[:, 0:BL], w1vq_t[:, 2 * kp:2 * kp + 2, fs],
                           x2q_t[:, 2 * kp:2 * kp + 2, :],
                           start=(kp == 0), stop=(kp == K1P // 2 - 1),
                           perf_mode=DR)
                    for si in range(2):
                        ss = slice(fo * FD + si * BL, fo * FD + (si + 1) * BL)
                        nc.scalar.activation(sig1e_t[:, ss], var_ps[:, 0:BL],
                                             AF.Sqrt, bias=eps12_t[:])
                    t_l = {}
                    for p in range(NP):
                        sl = slice(p * FD, (p + 1) * FD)
                        t_l[p] = tp.tile([128, FD], BF16, tag="t", name="t",
                                         bufs=3)
                        if p < 3:
                            nc.gpsimd.tensor_tensor(t_l[p][:], sig1e_t[:, es],
                                                    e1_t[fo][:, sl], ALU.mult)
                        else:
                            nc.vector.tensor_tensor(t_l[p][:], sig1e_t[:, es],
                                                    e1_t[fo][:, sl], ALU.mult)
                    mu_b = (mu_ps[:, 0:BL].unsqueeze(1)
                            .broadcast_to((128, 2, BL)))
                    for p in range(NP):
                        sl = slice(p * FD, (p + 1) * FD)
                        nc.vector._custom_dve(
                            PRELU_ADD,
                            out=h1_t[fo][:, sl].rearrange(
                                "q (s n) -> q s n", s=2),
                            in0=mu_b, in1=t_l[p][:], s0=0.0, imm2=0.01)
                    nc.scalar.activation(hq1_t[fo // 2][:, fo % 2, 0:3 * FD],
                                         h1_t[fo][:, 0:3 * FD], AF.Square,
                                         bias=0.0,
                                         scale=float(SQ_SCALE ** 0.5))
                    nc.scalar.activation(
                        hq1_t[fo // 2][:, fo % 2, 3 * FD:FDA],
                        h1_t[fo][:, 3 * FD:FDA], AF.Square, bias=0.0,
                        scale=float(SQ_SCALE ** 0.5))

                e2_t = load_eps(e2_d)

                # ---------- hidden layers ----------
                VAR_PAIRS = ((0, 1), (2, 3), (4,))

                def hidden_layer(h_in, hq_in, e_t, wm_t, wvq_t, bmP_t, bv_t):
                    h_o = h_tiles()
                    hq_o = hq_tiles()
                    for fo in range(FO):
                        fs = slice(fo * 128, (fo + 1) * 128)
                        sig_t = tp.tile([128, FDA], F32, tag="sigf",
                                        name="sig", bufs=2)
                        # var matmuls first: two pairs share one 2-bank tile
                        for vg in VAR_PAIRS:
                            vt = ps.tile([128, 2 * FD], F32, tag="var2",
                                         name="var2", bufs=2)
                            for kp in range(2):
                                for j, p in enumerate(vg):
                                    mm(vt[:, j * FD:(j + 1) * FD],
                                       wvq_t[:, 2 * kp:2 * kp + 2, fs],
                                       hq_in[kp][:, :, p * FD:(p + 1) * FD],
                                       start=(kp == 0), stop=(kp == 1),
                                       perf_mode=DR)
                            w = len(vg) * FD
                            nc.scalar.activation(
                                sig_t[:, vg[0] * FD:vg[0] * FD + w],
                                vt[:, 0:w], AF.Sqrt, bias=bv_t[:, fo:fo + 1])
                        # mean matmuls p-outer (ldw-opt is off anyway), then
                        # the fused mult / prelu-add chain per pair
                        for p in range(NP):
                            sl = slice(p * FD, (p + 1) * FD)
                            mu_p = mu_ps_tile()
                            for k in range(KH):
                                mm(mu_p[:], wm_t[:, k, fs],
                                   h_in[k][:, sl],
                                   start=(k == 0), stop=(k == KH - 1))
                            t_p = tp.tile([128, FD], BF16, tag="t", name="t",
                                          bufs=3)
                            nc.gpsimd.tensor_tensor(
                                t_p[:], sig_t[:, sl], e_t[fo][:, sl], ALU.mult)
                            nc.vector._custom_dve(
                                PRELU_ADD, out=h_o[fo][:, sl], in0=t_p[:],
                                in1=mu_p[:], s0=bmP_t[:, fo:fo + 1],
                                imm2=0.01)
                        nc.scalar.activation(
                            hq_o[fo // 2][:, fo % 2, :], h_o[fo][:],
                            AF.Square, bias=0.0,
                            scale=float(SQ_SCALE ** 0.5))
                    return h_o, hq_o

                if dbg:
                    nc.sync.dma_start(dbg_sig1[:], sig1e_t[:])
                    for fo in range(FO):
                        nc.sync.dma_start(dbg_h1[fo], h1_t[fo][:])
                    for kp in range(2):
                        nc.sync.dma_start(
                            dbg_hq1[kp],
                            hq1_t[kp][:].rearrange("p a b -> p (a b)"))

                h2_t, hq2_t = hidden_layer(h1_t, hq1_t, e2_t, w2m_t, w2vq_t,
                                           b2mP_t, b2v_t)
                if dbg:
                    for fo in range(FO):
                        nc.sync.dma_start(dbg_h2[fo], h2_t[fo][:])
                e3_t = load_eps(e3_d)
                h3_t, hq3_t = hidden_layer(h2_t, hq2_t, e3_t, w3m_t, w3vq_t,
                                           b3mP_t, b3v_t)

                # ---------- L4 ----------
                sig4a = tp.tile([C, FDA], BF16, tag="sig4", name="sig4",
                                bufs=1)
                var4_l, mu4_l = {}, {}
                for p in range(NP):
                    sl = slice(p * FD, (p + 1) * FD)
                    var4_l[p] = var_ps_tile()
                    for kp in range(2):
                        mm(var4_l[p][:, 0:FD], w4vq_t[:, 2 * kp:2 * kp + 2, :],
                           hq3_t[kp][:, :, sl],
                           start=(kp == 0), stop=(kp == 1), perf_mode=DR)
                    nc.scalar.activation(sig4a[:, sl], var4_l[p][0:C, 0:FD],
                                         AF.Sqrt, bias=b4v_t[:])
                for p in range(NP):
                    sl = slice(p * FD, (p + 1) * FD)
                    mu4_l[p] = mu_ps_tile()
                    for k in range(KH):
                        mm(mu4_l[p][0:C, :], w4m_t[:, k, :], h3_t[k][:, sl],
                           start=(k == 0), stop=(k == KH - 1))
                    t4_t = tp.tile([C, FD], BF16, tag="t4", name="t4", bufs=2)
                    nc.vector.tensor_tensor(t4_t[:], sig4a[:, sl], e4_t[:, sl],
                                            ALU.mult)
                    nc.vector.tensor_tensor(u4_all[:, sl], t4_t[:],
                                            mu4_l[p][0:C, :], ALU.add)

            if dbg:
                nc.sync.dma_start(dbg_u4[:], u4_all[:])

            # ---------- Phase C: log-softmax (exp/ln table) ----------
            # all Exps first, then all Lns -> exactly two ACT table loads
            with tc.tile_pool(name="psC", bufs=1, space="PSUM") as psC:
                ets, sps = [], []
                for p in range(NP):
                    sl = slice(p * FD, (p + 1) * FD)
                    et = tp.tile([C, FD], F32R, tag="et", name="et", bufs=3)
                    nc.scalar.activation(et[:], u4_all[:, sl], AF.Exp,
                                         bias=b4m_t[:])
                    ets.append(et)
                for p in range(NP):
                    s_ps = psC.tile([1, FD], F32, tag="s", name="s", bufs=5)
                    mm(s_ps[:], ones10[:], ets[p][:], start=True, stop=True)
                    sps.append(s_ps)
                for p in range(NP):
                    sl = slice(p * FD, (p + 1) * FD)
                    lsem = lsem_t[p % 2]
                    nc.scalar.activation(lsem[0:1, :], sps[p][:], AF.Ln,
                                         bias=0.0)
                    lseb_ps = psC.tile([C, FD], F32, tag="lseb", name="lseb",
                                       bufs=2)
                    mm(lseb_ps[:], bneg_t[:], lsem[:], start=True, stop=True)
                    o_t = tp.tile([C, FD], F32, tag="o", name="o", bufs=2)
                    nc.vector.tensor_tensor(o_t[:], u4_all[:, sl], lseb_ps[:],
                                            ALU.subtract)
                    nc.sync.dma_start(out_d[:, sl], o_t[:])

    nc.compile()
    return nc


def _pick_wv(v, base):
    """Scale c ~ base such that typical v*c lands exactly on the fp8 grid.
    For constant-v layers this zeroes the systematic quantization bias."""
    f = np.float32
    pos = v[v > 0]
    if pos.size == 0:
        return f(base)
    vm = f(np.median(pos))
    t = vm * f(base)
    q = f(np.asarray(t, ml_dtypes.float8_e4m3fn).astype(np.float32))
    if q <= 0:
        return f(base)
    return f(q / vm)


def prepare_core_inputs(inputs):
    f = np.float32
    bf = ml_dtypes.bfloat16
    f8 = ml_dtypes.float8_e4m3fn
    x = np.asarray(inputs["inputs"], dtype=f)

    w1m_full = np.asarray(inputs["a1_mean"], f)
    w1m = np.zeros((K1 * 128, H), f)
    w1m[:D_IN] = w1m_full
    w1m = w1m.reshape(K1, 128, H).astype(bf)
    s1 = np.asarray(inputs["a1_dropout"], f) * np.asarray(inputs["a1_scale"], f)
    v1f = (s1 * s1).astype(f)
    wv1 = _pick_wv(v1f, 64.0)
    c1 = X2_SCALE * wv1                      # var_ps = c1 * var
    v1 = np.zeros((K1P * 128, H), f)
    v1[:D_IN] = v1f * wv1
    w1vq = np.ascontiguousarray(
        v1.reshape(K1P, 128, H).transpose(1, 0, 2)).astype(f8)

    def hidden_w(mean, scale, dropout):
        m = np.asarray(mean, f)
        sc = (np.asarray(dropout, f) * np.asarray(scale, f)).astype(f)
        v = sc * sc
        wv = _pick_wv(v[:H], 128.0)
        c = np.float32(SQ_SCALE) * wv        # var_ps = c * var
        wm = np.ascontiguousarray(
            m[:H].reshape(KH, 128, H).transpose(1, 0, 2)).astype(bf)
        wvq = np.ascontiguousarray(
            (v[:H] * wv).reshape(KH, 128, H).transpose(1, 0, 2)).astype(f8)
        bmP = np.ascontiguousarray(m[H].reshape(FO, 128).T).astype(f)
        bv = np.ascontiguousarray(
            ((v[H] + np.float32(1e-12)) * c).reshape(FO, 128).T).astype(f)
        return wm, wvq, bmP, bv, c

    w2m, w2vq, b2mP, b2v, c2 = hidden_w(inputs["a2_mean"], inputs["a2_scale"],
                                        inputs["a2_dropout"])
    w3m, w3vq, b3mP, b3v, c3 = hidden_w(inputs["a3_mean"], inputs["a3_scale"],
                                        inputs["a3_dropout"])

    m4 = np.asarray(inputs["a4_mean"], f)
    s4 = np.asarray(inputs["a4_scale"], f)
    v4 = (s4 * s4).astype(f)
    wv4 = _pick_wv(v4[:H], 128.0)
    c4 = np.float32(SQ_SCALE) * wv4
    w4m = np.ascontiguousarray(
        m4[:H].reshape(KH, 128, C).transpose(1, 0, 2)).astype(bf)
    w4vq_s = (v4[:H] * wv4).reshape(KH, 128, C)
    w4vq_p = np.zeros((KH, 128, 128), np.float32)
    w4vq_p[:, :, :C] = w4vq_s
    w4vq = np.ascontiguousarray(w4vq_p.transpose(1, 0, 2)).astype(f8)
    b4m = np.ascontiguousarray(m4[H].reshape(C, 1)).astype(f)
    b4v = np.ascontiguousarray(((v4[H] + np.float32(1e-12)) * c4)
                               .reshape(C, 1)).astype(f)
    bneg = np.empty((2, C), f)
    bneg[0] = 1.0
    bneg[1] = -m4[H]

    shared = dict(w1m=w1m, w1vq=w1vq, w2m=w2m, w2vq=w2vq, w3m=w3m, w3vq=w3vq,
                  w4m=w4m, w4vq=w4vq, b2mP=b2mP, b3mP=b3mP, b2v=b2v, b3v=b3v,
                  b4v=b4v, b4m=b4m, bneg=bneg,
                  ones_row_in=np.ones((1, FD), dtype=f),
                  ones10_in=np.ones((C, 1), dtype=f))

    eps = [np.asarray(inputs[f"eps{i}"], f) for i in (1, 2, 3, 4)]

    es1 = f(1.0 / np.sqrt(c1))
    es2 = f(1.0 / np.sqrt(c2))
    es3 = f(1.0 / np.sqrt(c3))
    es4 = f(1.0 / np.sqrt(c4))

    def eT(e, b0, es):
        # [S, B, H] -> [FO, 128, (p, si, b)], pre-divided by sqrt(var scale)
        ec = e[:, b0:b0 + BL, :] * es                  # [10, BL, 512]
        return np.ascontiguousarray(
            ec.reshape(NP, 2, BL, FO, 128).transpose(3, 4, 0, 1, 2)
            .reshape(FO, 128, FDA)).astype(bf)

    def e4T(e, b0, es):
        ec = e[:, b0:b0 + BL, :] * es                  # [10, BL, C]
        return np.ascontiguousarray(
            ec.reshape(NP, 2, BL, C).transpose(3, 0, 1, 2)
            .reshape(C, FDA)).astype(bf)

    in_maps = []
    for i in range(N_CORES):
        b0 = i * BL
        xT = np.zeros((K1 * 128, BL), dtype=f)
        xT[:D_IN] = x[b0:b0 + BL].T
        x2 = np.zeros((K1P * 128, BL), dtype=f)
        x2[:D_IN] = (x[b0:b0 + BL].T ** 2) * X2_SCALE
        m = dict(shared)
        m["xT"] = np.ascontiguousarray(xT.reshape(K1, 128, BL)).astype(bf)
        m["x2q"] = np.ascontiguousarray(
            x2.reshape(K1P, 128, BL).transpose(1, 0, 2)).astype(f8)
        m["e1"] = eT(eps[0], b0, es1)
        m["e2"] = eT(eps[1], b0, es2)
        m["e3"] = eT(eps[2], b0, es3)
        m["e4"] = e4T(eps[3], b0, es4)
        in_maps.append(m)
    return in_maps


def gather_output(results):
    out = np.empty((S, B, C), dtype=np.float32)
    for i, r in enumerate(results):
        oc = np.asarray(r["out"])  # [C, (p, si, b)]
        oc = oc.reshape(C, NP, 2, BL).transpose(1, 2, 3, 0).reshape(S, BL, C)
        out[:, i * BL:(i + 1) * BL, :] = oc
    return out


_CACHE = {}


def run(inputs, trace=False, **spmd_kwargs):
    key = "prog"
    if key not in _CACHE:
        _CACHE[key] = build_program()
    nc = _CACHE[key]
    in_maps = prepare_core_inputs(inputs)
    res = run_bass_kernel_spmd(nc, in_maps, list(range(N_CORES)), trace=trace,
                               **spmd_kwargs)
    return gather_output(res.results), res


def kernel(**inputs):
    out, _ = run(inputs, trace=False)
    return out


# revision 59
# speedup vs baseline: 1.0895x; 1.0023x over previous
"""Trainium2 Bass kernel for a Bayesian MLP (local reparameterization trick).

Reference computation (per sample s of S=10):
    h1 = leaky_relu(x @ W1m + sqrt(x^2 @ W1v + 1e-12) * eps1_s)         [B, 512]
    h2 = leaky_relu(h1a @ W2m + sqrt(h1a^2 @ W2v + 1e-12) * eps2_s)     (h1a = [h1, 1])
    h3 = leaky_relu(h2a @ W3m + sqrt(h2a^2 @ W3v + 1e-12) * eps3_s)
    out = log_softmax(h3a @ W4m + sqrt(h3a^2 @ W4v + 1e-12) * eps4_s)   [B, 10]

Distribution: data-parallel over the batch axis, B=2048 -> 8 cores x 256 rows.
Small variational parameters replicated on every core.

v2 design notes:
  * activations [feature on partitions, (pair, sample, batch) free]; all ten
    samples live in one free axis of 5*512 = 2560 per feature block
  * mean matmuls in bf16 (stationary reused across the 5 sample-pairs)
  * variance matmuls in fp8 e4m3 with DoubleRow (K=256 per pass):
    hq = 8*h^2 (fp8), wv' = 256*v (fp8), descaled inside the ACT sqrt
  * elementwise work spread over ACT/DVE/Pool; big [128, 2560] ops where
    PSUM granularity allows
  * log-softmax phase at the end (single activation-table switch); the
    mean bias of layer 4 rides the Exp bias and a [2,C] stationary trick
"""

import sys
import os

for _p in ("/opt/trn_rl_repo",):
    if _p not in sys.path and os.path.isdir(_p):
        sys.path.insert(0, _p)

import numpy as np
import ml_dtypes

import concourse.bass as bass
import concourse.bacc as bacc
import concourse.mybir as mybir
from concourse import tile
from concourse.bass_utils import run_bass_kernel_spmd

F32 = mybir.dt.float32
F32R = mybir.dt.float32r
BF16 = mybir.dt.bfloat16
FP8 = mybir.dt.float8e4
AF = mybir.ActivationFunctionType
ALU = mybir.AluOpType
DR = mybir.MatmulPerfMode.DoubleRow


def _register_prelu_add():
    """Fused u = in0 + in1 + s0; out = max(0.01*u, u) as ONE DVE op.

    Replaces the separate tensor add and prelu passes of the local
    reparameterization chain (in1 may live in PSUM)."""
    import concourse.dve_ops as D
    from concourse.dve_spec import Spec, Src0, Src1, C0, C2, maxx, lower
    from concourse.dve_uop import DveOpSpec

    name = "PRELU_ADD_ANT"
    if name in D._SUB_OPCODE_FOR_NAME:
        for o in D.OPS:
            if o.name == name:
                return o
    _b = Src0 + Src1 + C0
    spec = Spec(
        body=maxx(_b * C2, _b),
        reference=lambda in0, in1, s0, s1, imm2: np.maximum(
            (in0.astype(np.float32) + in1 + s0) * imm2,
            in0.astype(np.float32) + in1 + s0),
    )
    opcode = D._CUSTOM_DVE_ROW_BASE + len(D.OPS)
    assert opcode < 0x20
    shas = {}
    for ver in ("v3", "v4"):
        uops = lower(spec, ver=ver)
        shas[ver] = DveOpSpec(name=name, opcode=opcode, uops=uops,
                              rd1_en=True).sha(ver)
    op = D.DveOp(name, spec, subdim=False, uops_sha=shas)
    D.OPS.append(op)
    D._SUB_OPCODE_FOR_NAME[name] = opcode
    return op


PRELU_ADD = _register_prelu_add()

B, D_IN, H, C, S = 2048, 784, 512, 10, 10
N_CORES = 8
BL = B // N_CORES            # 256 rows per core
NP = S // 2                  # 5 sample-pairs
FD = 2 * BL                  # 512 free per pair
FDA = NP * FD                # 2560 free, all pairs
K1 = 7                       # 784 -> 7 chunks of 112... no: 896/128
K1P = 8                      # padded to 8 for fp8 DoubleRow pairing
KH = 4                       # 512/128
FO = 4
SQ_SCALE = 2.0               # hq = SQ_SCALE * h^2 (compile-time, ACT Square)
X2_SCALE = 4.0               # x2q = X2_SCALE * x^2
# The fp8 variance-weight scale WV is chosen at RUNTIME per layer (so that
# constant-v layers land exactly on the fp8 grid); the descale is folded into
# the host-side eps tensors and sqrt-bias APs, so the device sqrt has scale=1.

PAIR_GROUPS = ((0, 1, 2), (3, 4))


def build_program(dbg=False):
    nc = bacc.Bacc("TRN2", target_bir_lowering=False, debug=False)

    # ---- DRAM I/O (per core) ----
    xT_d = nc.dram_tensor("xT", [K1, 128, BL], BF16, kind="ExternalInput")
    x2q_d = nc.dram_tensor("x2q", [128, K1P, BL], FP8, kind="ExternalInput")
    w1m_d = nc.dram_tensor("w1m", [K1, 128, H], BF16, kind="ExternalInput")
    w1vq_d = nc.dram_tensor("w1vq", [128, K1P, H], FP8, kind="ExternalInput")
    w2m_d = nc.dram_tensor("w2m", [128, KH, H], BF16, kind="ExternalInput")
    w2vq_d = nc.dram_tensor("w2vq", [128, KH, H], FP8, kind="ExternalInput")
    w3m_d = nc.dram_tensor("w3m", [128, KH, H], BF16, kind="ExternalInput")
    w3vq_d = nc.dram_tensor("w3vq", [128, KH, H], FP8, kind="ExternalInput")
    w4m_d = nc.dram_tensor("w4m", [128, KH, C], BF16, kind="ExternalInput")
    w4vq_d = nc.dram_tensor("w4vq", [128, KH, 128], FP8, kind="ExternalInput")
    b2mP_d = nc.dram_tensor("b2mP", [128, FO], F32, kind="ExternalInput")
    b3mP_d = nc.dram_tensor("b3mP", [128, FO], F32, kind="ExternalInput")
    b2v_d = nc.dram_tensor("b2v", [128, FO], F32, kind="ExternalInput")
    b3v_d = nc.dram_tensor("b3v", [128, FO], F32, kind="ExternalInput")
    b4v_d = nc.dram_tensor("b4v", [C, 1], F32, kind="ExternalInput")
    b4m_d = nc.dram_tensor("b4m", [C, 1], F32, kind="ExternalInput")
    bneg_d = nc.dram_tensor("bneg", [2, C], F32R, kind="ExternalInput")
    e1_d = nc.dram_tensor("e1", [FO, 128, FDA], BF16, kind="ExternalInput")
    e2_d = nc.dram_tensor("e2", [FO, 128, FDA], BF16, kind="ExternalInput")
    e3_d = nc.dram_tensor("e3", [FO, 128, FDA], BF16, kind="ExternalInput")
    e4_d = nc.dram_tensor("e4", [C, FDA], BF16, kind="ExternalInput")
    ones_row_d = nc.dram_tensor("ones_row_in", [1, FD], F32R, kind="ExternalInput")
    ones10_d = nc.dram_tensor("ones10_in", [C, 1], F32R, kind="ExternalInput")
    out_d = nc.dram_tensor("out", [C, FDA], F32, kind="ExternalOutput")
    if dbg:
        dbg_sig1 = nc.dram_tensor("dbg_sig1", [128, FO * FD], F32,
                                  kind="ExternalOutput")
        dbg_mu1 = nc.dram_tensor("dbg_mu1", [128, FO * FD], BF16,
                                 kind="ExternalOutput")
        dbg_h1 = nc.dram_tensor("dbg_h1", [FO, 128, FDA], BF16,
                                kind="ExternalOutput")
        dbg_hq1 = nc.dram_tensor("dbg_hq1", [2, 128, 2 * FDA], FP8,
                                 kind="ExternalOutput")
        dbg_h2 = nc.dram_tensor("dbg_h2", [FO, 128, FDA], BF16,
                                kind="ExternalOutput")
        dbg_u4 = nc.dram_tensor("dbg_u4", [C, FDA], F32,
                                kind="ExternalOutput")

    def mm(out_ap, lhsT_ap, rhs_ap, start, stop, perf_mode=None):
        nc.tensor.matmul(out_ap, lhsT_ap, rhs_ap, start=start, stop=stop,
                         perf_mode=perf_mode)

    with tile.TileContext(nc) as tc:
        with (
            tc.tile_pool(name="wp", bufs=1) as wp,
            tc.tile_pool(name="sp", bufs=1) as sp,
            tc.tile_pool(name="hp", bufs=1) as hp,
            tc.tile_pool(name="ep", bufs=1) as ep,
            tc.tile_pool(name="tp", bufs=1) as tp,
        ):
            # ---- persistent weights ----
            w1m_t = [wp.tile([128, H], BF16, tag=f"w1m{k}", name=f"w1m{k}")
                     for k in range(K1)]
            w1vq_t = wp.tile([128, K1P, H], FP8, tag="w1vq", name="w1vq")
            xT_t = [wp.tile([128, BL], BF16, tag=f"xT{k}", name=f"xT{k}")
                    for k in range(K1)]
            x2q_t = wp.tile([128, K1P, BL], FP8, tag="x2q", name="x2q")
            w2m_t = wp.tile([128, KH, H], BF16, tag="w2m", name="w2m")
            w2vq_t = wp.tile([128, KH, H], FP8, tag="w2vq", name="w2vq")
            w3m_t = wp.tile([128, KH, H], BF16, tag="w3m", name="w3m")
            w3vq_t = wp.tile([128, KH, H], FP8, tag="w3vq", name="w3vq")
            w4m_t = wp.tile([128, KH, C], BF16, tag="w4m", name="w4m")
            w4vq_t = wp.tile([128, KH, 128], FP8, tag="w4vq", name="w4vq")
            b2mP_t = wp.tile([128, FO], F32, tag="b2mP", name="b2mP")
            b3mP_t = wp.tile([128, FO], F32, tag="b3mP", name="b3mP")
            b2v_t = wp.tile([128, FO], F32, tag="b2v", name="b2v")
            b3v_t = wp.tile([128, FO], F32, tag="b3v", name="b3v")
            b4v_t = wp.tile([C, 1], F32, tag="b4v", name="b4v")
            b4m_t = wp.tile([C, 1], F32, tag="b4m", name="b4m")
            bneg_t = wp.tile([2, C], F32R, tag="bneg", name="bneg")
            ones10 = wp.tile([C, 1], F32R, tag="ones10", name="ones10")
            eps12_t = wp.tile([128, 1], F32, tag="eps12", name="eps12")
            nc.vector.memset(eps12_t[:], 1e-12)

            # ---- persistent activations ----
            # si-duplicated layer-1 stats (so L1 ops need no broadcast reads)
            sig1e_t = sp.tile([128, FO * FD], F32, tag="sig1e", name="sig1e")
            u4_all = sp.tile([C, FDA], F32, tag="u4", name="u4")
            e4_t = sp.tile([C, FDA], BF16, tag="e4", name="e4")
            lsem_t = [sp.tile([2, FD], F32R, tag=f"lsem{i}", name=f"lsem{i}")
                      for i in range(2)]

            # ---- weight DMAs ----
            for k in range(K1):
                nc.sync.dma_start(w1m_t[k][:], w1m_d[k])
                nc.sync.dma_start(xT_t[k][:], xT_d[k])
            nc.sync.dma_start(w1vq_t[:], w1vq_d[:])
            nc.sync.dma_start(x2q_t[:], x2q_d[:])
            nc.sync.dma_start(w2m_t[:], w2m_d[:])
            nc.sync.dma_start(w2vq_t[:], w2vq_d[:])
            nc.sync.dma_start(w3m_t[:], w3m_d[:])
            nc.sync.dma_start(w3vq_t[:], w3vq_d[:])
            nc.sync.dma_start(w4m_t[:], w4m_d[:])
            nc.sync.dma_start(w4vq_t[:], w4vq_d[:])
            nc.sync.dma_start(b2mP_t[:], b2mP_d[:])
            nc.sync.dma_start(b3mP_t[:], b3mP_d[:])
            nc.sync.dma_start(b2v_t[:], b2v_d[:])
            nc.sync.dma_start(b3v_t[:], b3v_d[:])
            nc.sync.dma_start(b4v_t[:], b4v_d[:])
            nc.sync.dma_start(b4m_t[:], b4m_d[:])
            nc.sync.dma_start(bneg_t[:], bneg_d[:])
            nc.sync.dma_start(ones10[:], ones10_d[:])
            nc.sync.dma_start(e4_t[:], e4_d[:])
            for i in range(2):
                nc.sync.dma_start(lsem_t[i][1:2, :], ones_row_d[:])

            # eps tiles: tag per fo, double-buffered across layers
            def load_eps(e_d):
                ts = []
                for fo in range(FO):
                    t = ep.tile([128, FDA], BF16, tag=f"e{fo}", name=f"e{fo}",
                                bufs=2)
                    nc.sync.dma_start(t[:], e_d[fo])
                    ts.append(t)
                return ts

            e1_t = load_eps(e1_d)

            # h/hq tiles: tag per fo / kp, double-buffered across layers
            def h_tiles():
                return [hp.tile([128, FDA], BF16, tag=f"h{fo}", name=f"h{fo}",
                                bufs=2) for fo in range(FO)]

            def hq_tiles():
                return [hp.tile([128, 2, FDA], FP8, tag=f"hq{kp}",
                                name=f"hq{kp}", bufs=2) for kp in range(2)]

            with tc.tile_pool(name="ps", bufs=1, space="PSUM") as ps:
                def mu_ps_tile():
                    return ps.tile([128, FD], F32, tag="mu", name="mu", bufs=4)

                def var_ps_tile():
                    return ps.tile([128, 2 * FD], F32, tag="var2", name="var2",
                                   bufs=2)

                # ---------- Phase A + L1, interleaved per feature block -------
                # L1(fo)'s elementwise rides behind phase A's matmuls for the
                # later feature blocks, so the PE-idle L1 zone shrinks.
                h1_t = h_tiles()
                hq1_t = hq_tiles()
                for fo in range(FO):
                    fs = slice(fo * 128, (fo + 1) * 128)
                    es = slice(fo * FD, (fo + 1) * FD)
                    mu_ps = mu_ps_tile()
                    for k in range(K1):
                        mm(mu_ps[:, 0:BL], w1m_t[k][:, fs], xT_t[k][:],
                           start=(k == 0), stop=(k == K1 - 1))
                    var_ps = var_ps_tile()
                    for kp in range(K1P // 2):
                        mm(var_ps[:, 0:BL], w1vq_t[:, 2 * kp:2 * kp + 2, fs],
                           x2q_t[:, 2 * kp:2 * kp + 2, :],
                           start=(kp == 0), stop=(kp == K1P // 2 - 1),
                           perf_mode=DR)
                    for si in range(2):
                        ss = slice(fo * FD + si * BL, fo * FD + (si + 1) * BL)
                        nc.scalar.activation(sig1e_t[:, ss], var_ps[:, 0:BL],
                                             AF.Sqrt, bias=eps12_t[:])
                    t_l = {}
                    for p in range(NP):
                        sl = slice(p * FD, (p + 1) * FD)
                        t_l[p] = tp.tile([128, FD], BF16, tag="t", name="t",
                                         bufs=3)
                        if p < 3:
                            nc.gpsimd.tensor_tensor(t_l[p][:], sig1e_t[:, es],
                                                    e1_t[fo][:, sl], ALU.mult)
                        else:
                            nc.vector.tensor_tensor(t_l[p][:], sig1e_t[:, es],
                                                    e1_t[fo][:, sl], ALU.mult)
                    mu_b = (mu_ps[:, 0:BL].unsqueeze(1)
                            .broadcast_to((128, 2, BL)))
                    for p in range(NP):
                        sl = slice(p * FD, (p + 1) * FD)
                        nc.vector._custom_dve(
                            PRELU_ADD,
                            out=h1_t[fo][:, sl].rearrange(
                                "q (s n) -> q s n", s=2),
                            in0=mu_b, in1=t_l[p][:], s0=0.0, imm2=0.01)
                    nc.scalar.activation(hq1_t[fo // 2][:, fo % 2, 0:3 * FD],
                                         h1_t[fo][:, 0:3 * FD], AF.Square,
                                         bias=0.0,
                                         scale=float(SQ_SCALE ** 0.5))
                    nc.scalar.activation(
                        hq1_t[fo // 2][:, fo % 2, 3 * FD:FDA],
                        h1_t[fo][:, 3 * FD:FDA], AF.Square, bias=0.0,
                        scale=float(SQ_SCALE ** 0.5))

                e2_t = load_eps(e2_d)

                # ---------- hidden layers ----------
                VAR_PAIRS = ((0, 1), (2, 3), (4,))

                def hidden_layer(h_in, hq_in, e_t, wm_t, wvq_t, bmP_t, bv_t):
                    h_o = h_tiles()
                    hq_o = hq_tiles()
                    for fo in range(FO):
                        fs = slice(fo * 128, (fo + 1) * 128)
                        sig_t = tp.tile([128, FDA], F32, tag="sigf",
                                        name="sig", bufs=2)
                        # var matmuls first: two pairs share one 2-bank tile
                        for vg in VAR_PAIRS:
                            vt = ps.tile([128, 2 * FD], F32, tag="var2",
                                         name="var2", bufs=2)
                            for kp in range(2):
                                for j, p in enumerate(vg):
                                    mm(vt[:, j * FD:(j + 1) * FD],
                                       wvq_t[:, 2 * kp:2 * kp + 2, fs],
                                       hq_in[kp][:, :, p * FD:(p + 1) * FD],
                                       start=(kp == 0), stop=(kp == 1),
                                       perf_mode=DR)
                            w = len(vg) * FD
                            nc.scalar.activation(
                                sig_t[:, vg[0] * FD:vg[0] * FD + w],
                                vt[:, 0:w], AF.Sqrt, bias=bv_t[:, fo:fo + 1])
                        # mean matmuls p-outer (ldw-opt is off anyway), then
                        # the fused mult / prelu-add chain per pair
                        for p in range(NP):
                            sl = slice(p * FD, (p + 1) * FD)
                            mu_p = mu_ps_tile()
                            for k in range(KH):
                                mm(mu_p[:], wm_t[:, k, fs],
                                   h_in[k][:, sl],
                                   start=(k == 0), stop=(k == KH - 1))
                            t_p = tp.tile([128, FD], BF16, tag="t", name="t",
                                          bufs=3)
                            nc.gpsimd.tensor_tensor(
                                t_p[:], sig_t[:, sl], e_t[fo][:, sl], ALU.mult)
                            nc.vector._custom_dve(
                                PRELU_ADD, out=h_o[fo][:, sl], in0=t_p[:],
                                in1=mu_p[:], s0=bmP_t[:, fo:fo + 1],
                                imm2=0.01)
                        nc.scalar.activation(
                            hq_o[fo // 2][:, fo % 2, 0:3 * FD],
                            h_o[fo][:, 0:3 * FD], AF.Square, bias=0.0,
                            scale=float(SQ_SCALE ** 0.5))
                        nc.scalar.activation(
                            hq_o[fo // 2][:, fo % 2, 3 * FD:FDA],
                            h_o[fo][:, 3 * FD:FDA], AF.Square, bias=0.0,
                            scale=float(SQ_SCALE ** 0.5))
                    return h_o, hq_o

                if dbg:
                    nc.sync.dma_start(dbg_sig1[:], sig1e_t[:])
                    for fo in range(FO):
                        nc.sync.dma_start(dbg_h1[fo], h1_t[fo][:])
                    for kp in range(2):
                        nc.sync.dma_start(
                            dbg_hq1[kp],
                            hq1_t[kp][:].rearrange("p a b -> p (a b)"))

                h2_t, hq2_t = hidden_layer(h1_t, hq1_t, e2_t, w2m_t, w2vq_t,
                                           b2mP_t, b2v_t)
                if dbg:
                    for fo in range(FO):
                        nc.sync.dma_start(dbg_h2[fo], h2_t[fo][:])
                e3_t = load_eps(e3_d)
                h3_t, hq3_t = hidden_layer(h2_t, hq2_t, e3_t, w3m_t, w3vq_t,
                                           b3mP_t, b3v_t)

                # ---------- L4 ----------
                sig4a = tp.tile([C, FDA], BF16, tag="sig4", name="sig4",
                                bufs=1)
                var4_l, mu4_l = {}, {}
                for p in range(NP):
                    sl = slice(p * FD, (p + 1) * FD)
                    var4_l[p] = var_ps_tile()
                    for kp in range(2):
                        mm(var4_l[p][:, 0:FD], w4vq_t[:, 2 * kp:2 * kp + 2, :],
                           hq3_t[kp][:, :, sl],
                           start=(kp == 0), stop=(kp == 1), perf_mode=DR)
                    nc.scalar.activation(sig4a[:, sl], var4_l[p][0:C, 0:FD],
                                         AF.Sqrt, bias=b4v_t[:])
                for p in range(NP):
                    sl = slice(p * FD, (p + 1) * FD)
                    mu4_l[p] = mu_ps_tile()
                    for k in range(KH):
                        mm(mu4_l[p][0:C, :], w4m_t[:, k, :], h3_t[k][:, sl],
                           start=(k == 0), stop=(k == KH - 1))
                    t4_t = tp.tile([C, FD], BF16, tag="t4", name="t4", bufs=2)
                    nc.vector.tensor_tensor(t4_t[:], sig4a[:, sl], e4_t[:, sl],
                                            ALU.mult)
                    nc.vector.tensor_tensor(u4_all[:, sl], t4_t[:],
                                            mu4_l[p][0:C, :], ALU.add)

            if dbg:
                nc.sync.dma_start(dbg_u4[:], u4_all[:])

            # ---------- Phase C: log-softmax (exp/ln table) ----------
            # all Exps first, then all Lns -> exactly two ACT table loads
            with tc.tile_pool(name="psC", bufs=1, space="PSUM") as psC:
                ets, sps = [], []
                for p in range(NP):
                    sl = slice(p * FD, (p + 1) * FD)
                    et = tp.tile([C, FD], F32R, tag="et", name="et", bufs=3)
                    nc.scalar.activation(et[:], u4_all[:, sl], AF.Exp,
                                         bias=b4m_t[:])
                    ets.append(et)
                for p in range(NP):
                    s_ps = psC.tile([1, FD], F32, tag="s", name="s", bufs=5)
                    mm(s_ps[:], ones10[:], ets[p][:], start=True, stop=True)
                    sps.append(s_ps)
                for p in range(NP):
                    sl = slice(p * FD, (p + 1) * FD)
                    lsem = lsem_t[p % 2]
                    nc.scalar.activation(lsem[0:1, :], sps[p][:], AF.Ln,
                                         bias=0.0)
                    lseb_ps = psC.tile([C, FD], F32, tag="lseb", name="lseb",
                                       bufs=2)
                    mm(lseb_ps[:], bneg_t[:], lsem[:], start=True, stop=True)
                    o_t = tp.tile([C, FD], F32, tag="o", name="o", bufs=2)
                    nc.vector.tensor_tensor(o_t[:], u4_all[:, sl], lseb_ps[:],
                                            ALU.subtract)
                    nc.sync.dma_start(out_d[:, sl], o_t[:])

    nc.compile()
    return nc


def _pick_wv(v, base):
    """Scale c ~ base such that typical v*c lands exactly on the fp8 grid.
    For constant-v layers this zeroes the systematic quantization bias."""
    f = np.float32
    pos = v[v > 0]
    if pos.size == 0:
        return f(base)
    vm = f(np.median(pos))
    t = vm * f(base)
    q = f(np.asarray(t, ml_dtypes.float8_e4m3fn).astype(np.float32))
    if q <= 0:
        return f(base)
    return f(q / vm)


def prepare_core_inputs(inputs):
    f = np.float32
    bf = ml_dtypes.bfloat16
    f8 = ml_dtypes.float8_e4m3fn
    x = np.asarray(inputs["inputs"], dtype=f)

    w1m_full = np.asarray(inputs["a1_mean"], f)
    w1m = np.zeros((K1 * 128, H), f)
    w1m[:D_IN] = w1m_full
    w1m = w1m.reshape(K1, 128, H).astype(bf)
    s1 = np.asarray(inputs["a1_dropout"], f) * np.asarray(inputs["a1_scale"], f)
    v1f = (s1 * s1).astype(f)
    wv1 = _pick_wv(v1f, 64.0)
    c1 = X2_SCALE * wv1                      # var_ps = c1 * var
    v1 = np.zeros((K1P * 128, H), f)
    v1[:D_IN] = v1f * wv1
    w1vq = np.ascontiguousarray(
        v1.reshape(K1P, 128, H).transpose(1, 0, 2)).astype(f8)

    def hidden_w(mean, scale, dropout):
        m = np.asarray(mean, f)
        sc = (np.asarray(dropout, f) * np.asarray(scale, f)).astype(f)
        v = sc * sc
        wv = _pick_wv(v[:H], 128.0)
        c = np.float32(SQ_SCALE) * wv        # var_ps = c * var
        wm = np.ascontiguousarray(
            m[:H].reshape(KH, 128, H).transpose(1, 0, 2)).astype(bf)
        wvq = np.ascontiguousarray(
            (v[:H] * wv).reshape(KH, 128, H).transpose(1, 0, 2)).astype(f8)
        bmP = np.ascontiguousarray(m[H].reshape(FO, 128).T).astype(f)
        bv = np.ascontiguousarray(
            ((v[H] + np.float32(1e-12)) * c).reshape(FO, 128).T).astype(f)
        return wm, wvq, bmP, bv, c

    w2m, w2vq, b2mP, b2v, c2 = hidden_w(inputs["a2_mean"], inputs["a2_scale"],
                                        inputs["a2_dropout"])
    w3m, w3vq, b3mP, b3v, c3 = hidden_w(inputs["a3_mean"], inputs["a3_scale"],
                                        inputs["a3_dropout"])

    m4 = np.asarray(inputs["a4_mean"], f)
    s4 = np.asarray(inputs["a4_scale"], f)
    v4 = (s4 * s4).astype(f)
    wv4 = _pick_wv(v4[:H], 128.0)
    c4 = np.float32(SQ_SCALE) * wv4
    w4m = np.ascontiguousarray(
        m4[:H].reshape(KH, 128, C).transpose(1, 0, 2)).astype(bf)
    w4vq_s = (v4[:H] * wv4).reshape(KH, 128, C)
    w4vq_p = np.zeros((KH, 128, 128), np.float32)
    w4vq_p[:, :, :C] = w4vq_s
    w4vq = np.ascontiguousarray(w4vq_p.transpose(1, 0, 2)).astype(f8)
    b4m = np.ascontiguousarray(m4[H].reshape(C, 1)).astype(f)
    b4v = np.ascontiguousarray(((v4[H] + np.float32(1e-12)) * c4)
                               .reshape(C, 1)).astype(f)
    bneg = np.empty((2, C), f)
    bneg[0] = 1.0
    bneg[1] = -m4[H]

    shared = dict(w1m=w1m, w1vq=w1vq, w2m=w2m, w2vq=w2vq, w3m=w3m, w3vq=w3vq,
                  w4m=w4m, w4vq=w4vq, b2mP=b2mP, b3mP=b3mP, b2v=b2v, b3v=b3v,
                  b4v=b4v, b4m=b4m, bneg=bneg,
                  ones_row_in=np.ones((1, FD), dtype=f),
                  ones10_in=np.ones((C, 1), dtype=f))

    eps = [np.asarray(inputs[f"eps{i}"], f) for i in (1, 2, 3, 4)]

    es1 = f(1.0 / np.sqrt(c1))
    es2 = f(1.0 / np.sqrt(c2))
    es3 = f(1.0 / np.sqrt(c3))
    es4 = f(1.0 / np.sqrt(c4))

    def eT(e, b0, es):
        # [S, B, H] -> [FO, 128, (p, si, b)], pre-divided by sqrt(var scale)
        ec = e[:, b0:b0 + BL, :] * es                  # [10, BL, 512]
        return np.ascontiguousarray(
            ec.reshape(NP, 2, BL, FO, 128).transpose(3, 4, 0, 1, 2)
            .reshape(FO, 128, FDA)).astype(bf)

    def e4T(e, b0, es):
        ec = e[:, b0:b0 + BL, :] * es                  # [10, BL, C]
        return np.ascontiguousarray(
            ec.reshape(NP, 2, BL, C).transpose(3, 0, 1, 2)
            .reshape(C, FDA)).astype(bf)

    in_maps = []
    for i in range(N_CORES):
        b0 = i * BL
        xT = np.zeros((K1 * 128, BL), dtype=f)
        xT[:D_IN] = x[b0:b0 + BL].T
        x2 = np.zeros((K1P * 128, BL), dtype=f)
        x2[:D_IN] = (x[b0:b0 + BL].T ** 2) * X2_SCALE
        m = dict(shared)
        m["xT"] = np.ascontiguousarray(xT.reshape(K1, 128, BL)).astype(bf)
        m["x2q"] = np.ascontiguousarray(
            x2.reshape(K1P, 128, BL).transpose(1, 0, 2)).astype(f8)
        m["e1"] = eT(eps[0], b0, es1)
        m["e2"] = eT(eps[1], b0, es2)
        m["e3"] = eT(eps[2], b0, es3)
        m["e4"] = e4T(eps[3], b0, es4)
        in_maps.append(m)
    return in_maps


def gather_output(results):
    out = np.empty((S, B, C), dtype=np.float32)
    for i, r in enumerate(results):
        oc = np.asarray(r["out"])  # [C, (p, si, b)]
        oc = oc.reshape(C, NP, 2, BL).transpose(1, 2, 3, 0).reshape(S, BL, C)
        out[:, i * BL:(i + 1) * BL, :] = oc
    return out


_CACHE = {}


def run(inputs, trace=False, **spmd_kwargs):
    key = "prog"
    if key not in _CACHE:
        _CACHE[key] = build_program()
    nc = _CACHE[key]
    in_maps = prepare_core_inputs(inputs)
    res = run_bass_kernel_spmd(nc, in_maps, list(range(N_CORES)), trace=trace,
                               **spmd_kwargs)
    return gather_output(res.results), res


def kernel(**inputs):
    out, _ = run(inputs, trace=False)
    return out
